# revision 22
# baseline (speedup 1.0000x reference)
"""AtomPairEmbedding Trainium2 kernel — 8-core SPMD, row-parallel.

Strategy (hardcoded for N=8192, K=32+16, PAIR=128, LOCAL=256):
 - Host glue: stable-permute atoms so aa atoms occupy table cols [0, n_aa_pad)
   and small molecules [n_aa_pad, NT); pad each category with far-away
   sentinels to a multiple of 512. Shard rows (permuted order) 1024/core.
 - Device per 128-row block:
     * PE computes  -d2 = 2*dot - sq_i - sq_j  via a K=5 matmul over the
       full table (NT/512 tiles) into PSUM; ScalarE evacuates to SBUF (V).
     * VectorE: per-128-col-segment max8 (top-8 per segment), then a
       max8/match_replace ladder on the segment candidates per category,
       with max_index against V to recover global column ids.
     * Edge features (16 RBF + type + same_residue + other_chain) built with
       tiny [128,48] ops; no gathers: type/chain/residue features derive
       from column ranges (category-contiguous layout + sorted chains).
     * Feature tile transposed via PE, then feature-on-partition MLP:
       pair = LN(edge @ Wp); h = gelu(pair@W1+b1); mlp=h@W2 accumulated in
       PSUM over all 48 neighbours; local = LN(local_in @ Wl).
 - Host glue out: un-permute rows, map table cols back to original atom ids.
"""

import numpy as np

N = 8192
K_AA, K_SMOL = 32, 16
KTOT = K_AA + K_SMOL
PAIR = 128
LOCAL = 256
RBF_BINS = 16
EPS = 1e-5
NCORES = 8
RPC = N // NCORES          # rows per core
BLK = 128                  # rows per block
NBLK = RPC // BLK
NEG_BIG = -3.0e38

_CACHE = {}


def _ceil_to(x, m):
    return (x + m - 1) // m * m


def _build_graph(NT, NAAP, nblk=NBLK, rpc=RPC, use_gelu=True, stage=50):
    """Build the single-core Bass graph (SPMD across 8 cores)."""
    import concourse.bacc as bacc
    import concourse.mybir as mybir
    from concourse.tile import TileContext

    fp32 = mybir.dt.float32
    u32 = mybir.dt.uint32
    i32 = mybir.dt.int32
    AF = mybir.ActivationFunctionType
    OP = mybir.AluOpType

    global RPC, NBLK
    RPC_SAVE, NBLK_SAVE = RPC, NBLK
    RPC, NBLK = rpc, nblk
    NSEG = NT // 128               # segments of 128
    SEG_AA = NAAP // 128
    NT_TILES = NT // 512
    R_AA = K_AA // 8               # merge rounds aa
    R_SM = K_SMOL // 8

    nc = bacc.Bacc()

    # ---- DRAM parameters (per-core values via in_maps) ----
    pos_rows = nc.declare_dram_parameter("pos_rows", [5, RPC], fp32, isOutput=False)
    posT = nc.declare_dram_parameter("posT", [5, NT], fp32, isOutput=False)
    own_col = nc.declare_dram_parameter("own_col", [RPC], fp32, isOutput=False)
    ch_lo_aa = nc.declare_dram_parameter("ch_lo_aa", [RPC], fp32, isOutput=False)
    ch_hi_aa = nc.declare_dram_parameter("ch_hi_aa", [RPC], fp32, isOutput=False)
    ch_lo_sm = nc.declare_dram_parameter("ch_lo_sm", [RPC], fp32, isOutput=False)
    ch_hi_sm = nc.declare_dram_parameter("ch_hi_sm", [RPC], fp32, isOutput=False)
    aa_row = nc.declare_dram_parameter("aa_row", [RPC], fp32, isOutput=False)
    isaa_row = nc.declare_dram_parameter("isaa_row", [RPC], fp32, isOutput=False)
    Wp_d = nc.declare_dram_parameter("Wp", [32, PAIR], fp32, isOutput=False)
    W1_d = nc.declare_dram_parameter("W1", [PAIR, 2 * PAIR], fp32, isOutput=False)
    W2_d = nc.declare_dram_parameter("W2", [2 * PAIR, LOCAL], fp32, isOutput=False)
    Wl_d = nc.declare_dram_parameter("Wl", [288, PAIR], fp32, isOutput=False)
    ln1g_d = nc.declare_dram_parameter("ln1_g", [PAIR], fp32, isOutput=False)
    ln1b_d = nc.declare_dram_parameter("ln1_b", [PAIR], fp32, isOutput=False)
    ln2g_d = nc.declare_dram_parameter("ln2_g", [PAIR], fp32, isOutput=False)
    ln2b_d = nc.declare_dram_parameter("ln2_b", [PAIR], fp32, isOutput=False)
    b1_d = nc.declare_dram_parameter("b1", [2 * PAIR], fp32, isOutput=False)
    b2_d = nc.declare_dram_parameter("b2", [LOCAL], fp32, isOutput=False)
    io128_d = nc.declare_dram_parameter("iota128", [128], fp32, isOutput=False)
    io21_d = nc.declare_dram_parameter("iota21v", [32], fp32, isOutput=False)

    pair_out = nc.declare_dram_parameter(
        "pair_out", [PAIR, NBLK * 12 * 512], fp32, isOutput=True)
    local_out = nc.declare_dram_parameter("local_out", [PAIR, RPC], fp32, isOutput=True)
    nbr_out = nc.declare_dram_parameter("nbr_out", [RPC, KTOT], u32, isOutput=True)

    with TileContext(nc) as tc:
        with tc.tile_pool(name="setup", bufs=1) as setup, \
             tc.tile_pool(name="blk", bufs=2) as blk, \
             tc.tile_pool(name="vpool", bufs=2) as vpool, \
             tc.tile_pool(name="psA", bufs=3, space="PSUM") as psA, \
             tc.tile_pool(name="psS", bufs=2, space="PSUM") as psS, \
             tc.tile_pool(name="accps", bufs=1, space="PSUM") as accps:

            # ---------- constants ----------
            ones_r = setup.tile([1, 128], fp32, tag="ones_r")
            nc.vector.memset(ones_r, 1.0)
            ident = setup.tile([128, 128], fp32, tag="ident")
            iota_col_f = setup.tile([128, 1], fp32, tag="iocf")
            iota_row_f = setup.tile([128, 128], fp32, tag="iorf")
            nc.sync.dma_start(out=iota_col_f, in_=io128_d[:].rearrange("(p o) -> p o", o=1))
            nc.sync.dma_start(out=iota_row_f[0:1, :], in_=io128_d[:].rearrange("(o n) -> o n", o=1))
            iops = psA.tile([128, 512], fp32, tag="ps")
            nc.tensor.matmul(iops[:, 0:128], ones_r, iota_row_f[0:1, :], start=True, stop=True)
            nc.scalar.copy(iota_row_f, iops[:, 0:128])
            nc.vector.tensor_scalar(ident, iota_row_f, iota_col_f, None, OP.is_equal)

            ones_k = setup.tile([128, 1], fp32, tag="ones_k")       # reduce over 128 parts
            nc.vector.memset(ones_k, 1.0)
            epsT = setup.tile([128, 1], fp32, tag="epsT")
            nc.vector.memset(epsT, EPS)


            # ---------- weights / vectors ----------
            Wp4 = setup.tile([128, 4, PAIR], fp32, tag="Wp4")
            nc.vector.memset(Wp4, 0.0)
            nc.sync.dma_start(out=Wp4[0:32, 0, :], in_=Wp_d[:, :])
            W1 = setup.tile([128, 2 * PAIR], fp32, tag="W1")
            nc.sync.dma_start(out=W1, in_=W1_d[:, :])
            W2 = setup.tile([128, 2, LOCAL], fp32, tag="W2")
            nc.sync.dma_start(out=W2, in_=W2_d[:, :].rearrange("(t p) n -> p t n", p=128))
            Wl = setup.tile([128, 3, PAIR], fp32, tag="Wl")
            nc.sync.dma_start(out=Wl[:, 0, :], in_=Wl_d[0:128, :])
            nc.sync.dma_start(out=Wl[:, 1, :], in_=Wl_d[128:256, :])
            nc.sync.dma_start(out=Wl[0:32, 2, :], in_=Wl_d[256:288, :])

            ln1g = setup.tile([128, 1], fp32, tag="ln1g")
            ln1b = setup.tile([128, 1], fp32, tag="ln1b")
            ln2g = setup.tile([128, 1], fp32, tag="ln2g")
            ln2b = setup.tile([128, 1], fp32, tag="ln2b")
            nc.sync.dma_start(out=ln1g, in_=ln1g_d[:].rearrange("(p o) -> p o", o=1))
            nc.sync.dma_start(out=ln1b, in_=ln1b_d[:].rearrange("(p o) -> p o", o=1))
            nc.sync.dma_start(out=ln2g, in_=ln2g_d[:].rearrange("(p o) -> p o", o=1))
            nc.sync.dma_start(out=ln2b, in_=ln2b_d[:].rearrange("(p o) -> p o", o=1))
            b1 = setup.tile([128, 2], fp32, tag="b1")
            nc.sync.dma_start(out=b1, in_=b1_d[:].rearrange("(t p) -> p t", p=128))
            b2x = setup.tile([128, 2], fp32, tag="b2x")
            nc.sync.dma_start(out=b2x, in_=b2_d[:].rearrange("(t p) -> p t", p=128))
            nc.vector.tensor_scalar(b2x, b2x, float(KTOT), None, OP.mult)

            # center Wp / Wl columns (free axis) so LN mean-subtract is free
            wpm = setup.tile([32, 1], fp32, tag="wpm")
            nc.vector.tensor_reduce(wpm, Wp4[0:32, 0, :], mybir.AxisListType.X, OP.add)
            nc.vector.tensor_scalar(wpm, wpm, 1.0 / PAIR, None, OP.mult)
            nc.vector.tensor_scalar(Wp4[0:32, 0, :], Wp4[0:32, 0, :], wpm, None, OP.subtract)
            for q in range(1, 4):
                nc.sync.dma_start(out=Wp4[q * 32:(q + 1) * 32, q, :], in_=Wp4[0:32, 0, :])
            for t in range(3):
                pr = 128 if t < 2 else 32
                wlm = setup.tile([128, 1], fp32, tag="wlm")
                nc.vector.tensor_reduce(wlm[:pr], Wl[:pr, t, :], mybir.AxisListType.X, OP.add)
                nc.vector.tensor_scalar(wlm[:pr], wlm[:pr], 1.0 / PAIR, None, OP.mult)
                nc.vector.tensor_scalar(Wl[:pr, t, :], Wl[:pr, t, :], wlm[:pr], None, OP.subtract)

            # extras table [32, RPC]: row0 = is_aa, rows 1..21 = one_hot(aa)
            ex_all = setup.tile([32, RPC], fp32, tag="ex_all")
            nc.vector.memset(ex_all, 0.0)
            aarow_sb = setup.tile([1, RPC], fp32, tag="aarow_sb")
            nc.sync.dma_start(out=aarow_sb, in_=aa_row[:].rearrange("(o n) -> o n", o=1))
            for t0 in range(0, RPC, 512):
                w = min(512, RPC - t0)
                sl = slice(t0, t0 + w)
                bps = psA.tile([128, 512], fp32, tag="ps")
                nc.tensor.matmul(bps[0:21, 0:w], ones_r[:, 0:21], aarow_sb[:, sl],
                                 start=True, stop=True)
                nc.scalar.copy(ex_all[0:21, sl], bps[0:21, 0:w])
            nc.sync.dma_start(out=ex_all[21:22, :], in_=isaa_row[:].rearrange("(o n) -> o n", o=1))
            iota21 = setup.tile([32, 1], fp32, tag="io21")
            nc.sync.dma_start(out=iota21, in_=io21_d[:].rearrange("(p o) -> p o", o=1))
            nc.vector.tensor_scalar(ex_all[0:21, :], ex_all[0:21, :], iota21[0:21], None, OP.is_equal)

            # per-row scalars, laid out [128, NBLK]
            rowsc = {}
            for nm, dv in (("own", own_col), ("laa", ch_lo_aa), ("haa", ch_hi_aa),
                           ("lsm", ch_lo_sm), ("hsm", ch_hi_sm)):
                t_ = setup.tile([128, NBLK], fp32, tag="rs_" + nm)
                nc.sync.dma_start(out=t_, in_=dv[:].rearrange("(b p) -> p b", p=128))
                rowsc[nm] = t_

            # ---------- rhs table [5, NT]: 2-> posT, ones, sq ----------
            rhs = setup.tile([5, NT], fp32, tag="rhs")
            nc.sync.dma_start(out=rhs, in_=posT[:, :])
            possq = vpool.tile([128, NT], fp32, tag="V")   # reuse V storage
            nc.vector.tensor_mul(possq[0:3, :], rhs[0:3, :], rhs[0:3, :])
            ones3 = setup.tile([3, 1], fp32, tag="ones3")
            nc.vector.memset(ones3, 1.0)
            sqtab_t = vpool.tile([128, NT], fp32, tag="V")
            sqtab = sqtab_t[0:1, :]
            for t in range(NT_TILES):
                sl = slice(t * 512, (t + 1) * 512)
                sq_ps = psA.tile([128, 512], fp32, tag="ps")
                nc.tensor.matmul(sq_ps[0:1, :], ones3, possq[0:3, sl], start=True, stop=True)
                nc.scalar.copy(sqtab[:, sl], sq_ps[0:1, :])
            nc.sync.dma_start(out=rhs[4:5, :], in_=sqtab)

            # ---------- lhsT [5, RPC]: 2*pos, -sq_i, -1 ----------
            lhsT = setup.tile([5, RPC], fp32, tag="lhsT")
            nc.sync.dma_start(out=lhsT, in_=pos_rows[:, :])
            nc.vector.tensor_scalar(lhsT[0:3, :], lhsT[0:3, :], 2.0, None, OP.mult)
            rsq = setup.tile([3, RPC], fp32, tag="rsq")
            nc.sync.dma_start(out=rsq, in_=pos_rows[0:3, :])
            nc.vector.tensor_mul(rsq, rsq, rsq)
            negones3 = setup.tile([3, 1], fp32, tag="negones3")
            nc.vector.memset(negones3, -1.0)
            sqi_tmp = setup.tile([1, RPC], fp32, tag="sqi_tmp")
            for t0 in range(0, RPC, 512):
                w = min(512, RPC - t0)
                sl = slice(t0, t0 + w)
                sq_ps2 = psA.tile([128, 512], fp32, tag="ps")
                nc.tensor.matmul(sq_ps2[0:1, 0:w], negones3, rsq[:, sl], start=True, stop=True)
                nc.scalar.copy(sqi_tmp[:, sl], sq_ps2[0:1, 0:w])
            # move row (partition 0) to partition 3 of lhsT via sbuf-sbuf DMA
            nc.sync.dma_start(out=lhsT[3:4, :], in_=sqi_tmp)

            # =====================================================
            # per-block processing
            # =====================================================
            for b in range(NBLK):
                lb = lhsT[:, b * BLK:(b + 1) * BLK]

                # ---- -d2 + evacuation ----
                V = vpool.tile([128, NT], fp32, tag="V")
                for t in range(NT_TILES):
                    sl = slice(t * 512, (t + 1) * 512)
                    ps = psA.tile([128, 512], fp32, tag="ps")
                    nc.tensor.matmul(ps, lb, rhs[:, sl], start=True, stop=True)
                    nc.scalar.copy(V[:, sl], ps)

                if stage < 2:
                    continue
                # ---- seg top8 ----
                S = blk.tile([128, NSEG * 8], fp32, tag="S")
                for s in range(NSEG):
                    nc.vector.max(S[:, s * 8:(s + 1) * 8], V[:, s * 128:(s + 1) * 128])

                # ---- merge ladders ----
                Wv = blk.tile([128, KTOT], fp32, tag="Wv")
                Gu = blk.tile([128, KTOT], u32, tag="Gu")
                Saa = S[:, :SEG_AA * 8]
                Ssm = S[:, SEG_AA * 8:NSEG * 8]
                Vaa = V[:, :NAAP]
                Vsm = V[:, NAAP:NT]
                for r in range(R_AA):
                    w8 = Wv[:, r * 8:(r + 1) * 8]
                    nc.vector.max(w8, Saa)
                    nc.vector.max_index(Gu[:, r * 8:(r + 1) * 8], w8, Vaa)
                    if r + 1 < R_AA:
                        nc.vector.match_replace(Saa, w8, Saa, NEG_BIG)
                for r in range(R_SM):
                    w8 = Wv[:, K_AA + r * 8:K_AA + (r + 1) * 8]
                    nc.vector.max(w8, Ssm)
                    nc.vector.max_index(Gu[:, K_AA + r * 8:K_AA + (r + 1) * 8], w8, Vsm)
                    if r + 1 < R_SM:
                        nc.vector.match_replace(Ssm, w8, Ssm, NEG_BIG)

                nc.sync.dma_start(out=nbr_out[b * BLK:(b + 1) * BLK, :], in_=Gu)

                if stage < 3:
                    continue
                # ---- features [128, 48] ----
                Gf = blk.tile([128, KTOT], fp32, tag="Gf")
                nc.vector.tensor_copy(Gf, Gu)
                nc.vector.tensor_scalar(Gf[:, K_AA:], Gf[:, K_AA:], float(NAAP), None, OP.add)

                d2p = blk.tile([128, KTOT], fp32, tag="d2p")
                nc.vector.tensor_scalar(d2p, Wv, -1.0, 1e-12, OP.mult, OP.max)
                dd = blk.tile([128, KTOT], fp32, tag="dd")
                nc.scalar.activation(dd, d2p, AF.Sqrt)

                sc = blk.tile([128, KTOT], fp32, tag="sc")
                m2 = blk.tile([128, KTOT], fp32, tag="m2")
                bsl = slice(b, b + 1)
                nc.vector.tensor_scalar(sc[:, :K_AA], Gf[:, :K_AA], rowsc["laa"][:, bsl], None, OP.is_ge)
                nc.vector.tensor_scalar(m2[:, :K_AA], Gf[:, :K_AA], rowsc["haa"][:, bsl], None, OP.is_lt)
                nc.vector.tensor_scalar(sc[:, K_AA:], Gf[:, K_AA:], rowsc["lsm"][:, bsl], None, OP.is_ge)
                nc.vector.tensor_scalar(m2[:, K_AA:], Gf[:, K_AA:], rowsc["hsm"][:, bsl], None, OP.is_lt)
                nc.vector.tensor_mul(sc, sc, m2)
                oc = blk.tile([128, KTOT], fp32, tag="oc")
                nc.vector.tensor_scalar(oc, sc, -1.0, 1.0, OP.mult, OP.add)
                sr = blk.tile([128, KTOT], fp32, tag="sr")
                nc.vector.tensor_scalar(sr, Gf, rowsc["own"][:, bsl], None, OP.is_equal)

                # rbf: scratch r-major [16 x 48], squared, exp into F strided
                F = blk.tile([128, KTOT * 32], fp32, tag="F")
                nc.vector.memset(F, 0.0)
                Rt = blk.tile([128, RBF_BINS * KTOT], fp32, tag="Rt")
                sig = (22.0 - 2.0) / RBF_BINS
                for r_ in range(RBF_BINS):
                    c_r = 2.0 + r_ * (20.0 / (RBF_BINS - 1))
                    nc.vector.tensor_scalar(
                        Rt[:, r_ * KTOT:(r_ + 1) * KTOT], dd, -c_r, None, OP.add)
                nc.vector.tensor_mul(Rt, Rt, Rt)
                Fv = F[:, :].rearrange("p (k f) -> p k f", f=32)
                Rv = Rt[:, :].rearrange("p (f k) -> p k f", k=KTOT)
                nc.scalar.activation(Fv[:, :, 0:RBF_BINS], Rv, AF.Exp,
                                     scale=-1.0 / (sig * sig))
                nc.vector.memset(Fv[:, 0:K_AA, 16:17], 1.0)
                nc.vector.tensor_copy(Fv[:, :, 17:18], sr[:, :].rearrange("p (k o) -> p k o", o=1))
                nc.vector.tensor_copy(Fv[:, :, 18:19], oc[:, :].rearrange("p (k o) -> p k o", o=1))

                if stage < 35:
                    continue
                # ---- transpose F -> FT [128, 12, 128] ----
                FT = blk.tile([128, 12, 128], fp32, tag="FT")
                for j in range(12):
                    tps = psA.tile([128, 512], fp32, tag="ps")
                    nc.tensor.transpose(tps[:, 0:128], F[:, j * 128:(j + 1) * 128], ident)
                    nc.scalar.copy(FT[:, j, :], tps[:, 0:128])

                if stage < 40:
                    continue
                # ---- MLP over 12 j-tiles ----
                mlp_ps0 = accps.tile([128, 512], fp32, tag="mlpa")
                mlp_ps1 = accps.tile([128, 512], fp32, tag="mlpb")
                for j in range(12):
                    pairps = psA.tile([128, 512], fp32, tag="ps")
                    for kk in range(4):
                        nc.tensor.matmul(
                            pairps[:, kk * 128:(kk + 1) * 128],
                            Wp4[:, kk, :],
                            FT[:, j, :], start=True, stop=True)
                    # LN1
                    sqt = blk.tile([128, 512], fp32, tag="sqt")
                    nc.scalar.activation(sqt, pairps, AF.Square)
                    ssq = psS.tile([1, 512], fp32, tag="small")
                    nc.tensor.matmul(ssq, ones_k, sqt, start=True, stop=True)
                    sgt = blk.tile([1, 512], fp32, tag="sgt")
                    nc.scalar.activation(sgt, ssq, AF.Sqrt, scale=1.0 / PAIR, bias=epsT[0:1, :])
                    rsg = blk.tile([1, 512], fp32, tag="rsg")
                    nc.vector.reciprocal(rsg, sgt)
                    rep = blk.tile([128, 512], fp32, tag="rep")
                    nc.gpsimd.partition_broadcast(rep, rsg)
                    pair_sb = blk.tile([128, 512], fp32, tag="pair_sb")
                    nc.vector.scalar_tensor_tensor(
                        pair_sb, pairps, ln1g, rep, OP.mult, OP.mult)
                    nc.vector.tensor_scalar(pair_sb, pair_sb, ln1b, None, OP.add)
                    nc.sync.dma_start(
                        out=pair_out[:, (b * 12 + j) * 512:(b * 12 + j + 1) * 512],
                        in_=pair_sb)
                    # W1 + gelu
                    hs = blk.tile([128, 2, 512], fp32, tag="hs")
                    for t in range(2):
                        hps = psA.tile([128, 512], fp32, tag="ps")
                        nc.tensor.matmul(hps, W1[:, t * 128:(t + 1) * 128], pair_sb,
                                         start=True, stop=True)
                        gf = AF.Gelu_apprx_tanh if use_gelu else AF.Square
                        nc.scalar.activation(hs[:, t, :], hps, gf,
                                             bias=b1[:, t:t + 1])
                    # W2 accumulate over j
                    for t, mps in ((0, mlp_ps0), (1, mlp_ps1)):
                        for s in range(2):
                            nc.tensor.matmul(
                                mps, W2[:, s, t * 128:(t + 1) * 128], hs[:, s, :],
                                start=(j == 0 and s == 0), stop=(j == 11 and s == 1),
                                skip_group_check=True)

                if stage < 50:
                    continue
                # ---- reduce over kk; + 48*b2 ----
                pw = blk.tile([128, 2, 128], fp32, tag="pw")
                for t, mps in ((0, mlp_ps0), (1, mlp_ps1)):
                    nc.vector.tensor_reduce(
                        pw[:, t, :], mps[:, :].rearrange("p (k i) -> p i k", k=4),
                        mybir.AxisListType.X, OP.add)
                    nc.vector.tensor_scalar(pw[:, t, :], pw[:, t, :], b2x[:, t:t + 1], None, OP.add)

                # ---- Wl matmuls + LN2 ----
                rsl = slice(b * BLK, (b + 1) * BLK)
                y2 = psS.tile([128, 128], fp32, tag="small")
                nc.tensor.matmul(y2, Wl[:, 0, :], pw[:, 0, :], start=True, stop=False)
                nc.tensor.matmul(y2, Wl[:, 1, :], pw[:, 1, :], start=False, stop=False)
                nc.tensor.matmul(y2, Wl[0:32, 2, :], ex_all[:, rsl], start=False, stop=True)
                sq2 = blk.tile([128, 128], fp32, tag="sq2")
                nc.scalar.activation(sq2, y2, AF.Square)
                ssq2 = psS.tile([1, 128], fp32, tag="small")
                nc.tensor.matmul(ssq2, ones_k, sq2, start=True, stop=True)
                sg2 = blk.tile([1, 128], fp32, tag="sg2")
                nc.scalar.activation(sg2, ssq2, AF.Sqrt, scale=1.0 / PAIR, bias=epsT[0:1, :])
                rs2 = blk.tile([1, 128], fp32, tag="rs2")
                nc.vector.reciprocal(rs2, sg2)
                rep2_ps = psA.tile([128, 512], fp32, tag="ps")
                nc.tensor.matmul(rep2_ps[:, 0:128], ones_r, rs2, start=True, stop=True)
                loc = blk.tile([128, 128], fp32, tag="loc")
                nc.vector.tensor_scalar(loc, y2, ln2g, None, OP.mult)
                nc.vector.tensor_mul(loc, loc, rep2_ps[:, 0:128])
                nc.vector.tensor_scalar(loc, loc, ln2b, None, OP.add)
                nc.sync.dma_start(out=local_out[:, rsl], in_=loc)

    nc.finalize()
    RPC, NBLK = RPC_SAVE, NBLK_SAVE
    return nc


def _host_prep(inputs):
    pos = np.asarray(inputs["positions"])[:, 1].astype(np.float32)      # [N,3]
    is_aa = np.asarray(inputs["is_aa_int"]).astype(bool)
    chain = np.asarray(inputs["chain_index"]).astype(np.int64)

    aa_idx = np.where(is_aa)[0]
    sm_idx = np.where(~is_aa)[0]
    n_aa, n_sm = len(aa_idx), len(sm_idx)
    NAAP = _ceil_to(n_aa, 512)
    NSMP = _ceil_to(n_sm, 512)
    NT = NAAP + NSMP

    perm = np.concatenate([aa_idx, sm_idx])            # row r -> orig atom
    col2orig = np.full(NT, 0, np.int64)
    col2orig[:n_aa] = aa_idx
    col2orig[NAAP:NAAP + n_sm] = sm_idx

    table = np.full((NT, 3), 1.0e4, np.float32)
    table[:n_aa] = pos[aa_idx]
    table[NAAP:NAAP + n_sm] = pos[sm_idx]
    npad_a = NAAP - n_aa
    table[n_aa:NAAP, 0] += 64.0 * np.arange(1, npad_a + 1, dtype=np.float32)
    npad_s = NSMP - n_sm
    table[NAAP + n_sm:, 1] += 64.0 * np.arange(1, npad_s + 1, dtype=np.float32)

    row_col = np.empty(N, np.float32)                  # row -> its table col
    row_col[:n_aa] = np.arange(n_aa)
    row_col[n_aa:] = NAAP + np.arange(n_sm)

    ch_of_row = chain[perm]
    ch_aa = chain[aa_idx]; ch_sm = chain[sm_idx]
    nch = int(chain.max()) + 1
    aa_st = np.searchsorted(ch_aa, np.arange(nch + 1))
    sm_st = np.searchsorted(ch_sm, np.arange(nch + 1))
    c = ch_of_row
    lo_aa = aa_st[c].astype(np.float32); hi_aa = aa_st[c + 1].astype(np.float32)
    lo_sm = (NAAP + sm_st[c]).astype(np.float32); hi_sm = (NAAP + sm_st[c + 1]).astype(np.float32)

    return dict(NT=NT, NAAP=NAAP, perm=perm, col2orig=col2orig, table=table,
                row_col=row_col, lo_aa=lo_aa, hi_aa=hi_aa, lo_sm=lo_sm, hi_sm=hi_sm)


def kernel(_trace=False, _trace_dir=None, **inputs):
    import concourse.bass_utils as bass_utils

    hp = _host_prep(inputs)
    NT, NAAP = hp["NT"], hp["NAAP"]
    perm = hp["perm"]

    key = (NT, NAAP)
    if key not in _CACHE:
        _CACHE[key] = _build_graph(NT, NAAP)
    nc = _CACHE[key]

    NTv = hp["NT"]
    posT = np.concatenate([
        np.ascontiguousarray(hp["table"].T),
        np.ones((1, NTv), np.float32),
        np.zeros((1, NTv), np.float32)], axis=0)               # [5, NT]
    pos = np.asarray(inputs["positions"])[:, 1].astype(np.float32)
    aa = np.asarray(inputs["aa"]).astype(np.float32)
    isaa_f = np.asarray(inputs["is_aa_int"]).astype(np.float32)

    Wp_p = np.zeros((32, PAIR), np.float32)
    Wp_p[:19] = np.asarray(inputs["Wp"], np.float32)
    Wl_full = np.asarray(inputs["Wl"], np.float32)
    Wl_p = np.zeros((288, PAIR), np.float32)
    Wl_p[:256] = Wl_full[:256]
    Wl_p[256:277] = Wl_full[257:278]      # one_hot rows first
    Wl_p[277] = Wl_full[256]              # then is_aa row

    in_maps = []
    for c in range(NCORES):
        rs = slice(c * RPC, (c + 1) * RPC)
        rows = perm[rs]
        in_maps.append({
            "pos_rows": np.concatenate([
                np.ascontiguousarray(pos[rows].T),
                np.zeros((1, RPC), np.float32),
                np.full((1, RPC), -1.0, np.float32)], axis=0),
            "posT": posT,
            "own_col": hp["row_col"][rs],
            "ch_lo_aa": hp["lo_aa"][rs], "ch_hi_aa": hp["hi_aa"][rs],
            "ch_lo_sm": hp["lo_sm"][rs], "ch_hi_sm": hp["hi_sm"][rs],
            "aa_row": aa[rows], "isaa_row": isaa_f[rows],
            "Wp": Wp_p, "W1": np.asarray(inputs["W1"], np.float32),
            "W2": np.asarray(inputs["W2"], np.float32), "Wl": Wl_p,
            "ln1_g": np.asarray(inputs["ln1_g"], np.float32),
            "ln1_b": np.asarray(inputs["ln1_b"], np.float32),
            "ln2_g": np.asarray(inputs["ln2_g"], np.float32),
            "ln2_b": np.asarray(inputs["ln2_b"], np.float32),
            "b1": np.asarray(inputs["b1"], np.float32),
            "b2": np.asarray(inputs["b2"], np.float32),
            "iota128": np.arange(128, dtype=np.float32),
            "iota21v": np.concatenate([np.arange(21), np.zeros(11)]).astype(np.float32),
        })

    kw = {}
    if _trace:
        kw = dict(trace=True, tmpdir=_trace_dir)
    res = bass_utils.run_bass_kernel_spmd(nc, in_maps, core_ids=list(range(NCORES)), **kw)
    if _trace:
        print("HW exec time:", res.exec_time_ns, "ns")
    results = res.results

    # ---- host unshard / unpermute ----
    local = np.empty((N, PAIR), np.float32)
    pair = np.empty((N, KTOT, PAIR), np.float32)
    nbrs = np.empty((N, KTOT), np.int32)

    for c in range(NCORES):
        r = results[c]
        rows = perm[c * RPC:(c + 1) * RPC]
        local[rows] = np.asarray(r["local_out"]).T
        po = np.asarray(r["pair_out"]).reshape(PAIR, NBLK, 12, 4, 128)
        po = po.transpose(1, 4, 2, 3, 0).reshape(RPC, KTOT, PAIR)
        pair[rows] = po
        nb = np.asarray(r["nbr_out"]).astype(np.int64)
        nb[:, K_AA:] += NAAP
        nbrs[rows] = hp["col2orig"][nb]

    mask = np.asarray(inputs["all_atom_mask"])[:, 1] > 0
    return (local, pair, nbrs.astype(np.int32), np.asarray(mask))


# revision 27
# speedup vs baseline: 1.0521x; 1.0521x over previous
"""AtomPairEmbedding Trainium2 kernel — 8-core SPMD, row-parallel.

Strategy (hardcoded for N=8192, K=32+16, PAIR=128, LOCAL=256):
 - Host glue: stable-permute atoms so aa atoms occupy table cols [0, n_aa_pad)
   and small molecules [n_aa_pad, NT); pad each category with far-away
   sentinels to a multiple of 512. Shard rows (permuted order) 1024/core.
 - Device per 128-row block:
     * PE computes  -d2 = 2*dot - sq_i - sq_j  via a K=5 matmul over the
       full table (NT/512 tiles) into PSUM; ScalarE evacuates to SBUF (V).
     * VectorE: per-128-col-segment max8 (top-8 per segment), then a
       max8/match_replace ladder on the segment candidates per category,
       with max_index against V to recover global column ids.
     * Edge features (16 RBF + type + same_residue + other_chain) built with
       tiny [128,48] ops; no gathers: type/chain/residue features derive
       from column ranges (category-contiguous layout + sorted chains).
     * Feature tile transposed via PE, then feature-on-partition MLP:
       pair = LN(edge @ Wp); h = gelu(pair@W1+b1); mlp=h@W2 accumulated in
       PSUM over all 48 neighbours; local = LN(local_in @ Wl).
 - Host glue out: un-permute rows, map table cols back to original atom ids.
"""

import numpy as np

N = 8192
K_AA, K_SMOL = 32, 16
KTOT = K_AA + K_SMOL
PAIR = 128
LOCAL = 256
RBF_BINS = 16
EPS = 1e-5
NCORES = 8
RPC = N // NCORES          # rows per core
BLK = 128                  # rows per block
NBLK = RPC // BLK
NEG_BIG = -3.0e38

_CACHE = {}


def _ceil_to(x, m):
    return (x + m - 1) // m * m


def _build_graph(NT, NAAP, nblk=NBLK, rpc=RPC, use_gelu=True, stage=50):
    """Build the single-core Bass graph (SPMD across 8 cores)."""
    import concourse.bacc as bacc
    import concourse.mybir as mybir
    from concourse.tile import TileContext

    fp32 = mybir.dt.float32
    bf16 = mybir.dt.bfloat16
    u32 = mybir.dt.uint32
    i32 = mybir.dt.int32
    AF = mybir.ActivationFunctionType
    OP = mybir.AluOpType

    global RPC, NBLK
    RPC_SAVE, NBLK_SAVE = RPC, NBLK
    RPC, NBLK = rpc, nblk
    NSEG = NT // 128               # segments of 128
    SEG_AA = NAAP // 128
    NT_TILES = NT // 512
    R_AA = K_AA // 8               # merge rounds aa
    R_SM = K_SMOL // 8

    nc = bacc.Bacc()

    # ---- DRAM parameters (per-core values via in_maps) ----
    pos_rows = nc.declare_dram_parameter("pos_rows", [5, RPC], fp32, isOutput=False)
    posT = nc.declare_dram_parameter("posT", [5, NT], fp32, isOutput=False)
    own_col = nc.declare_dram_parameter("own_col", [RPC], fp32, isOutput=False)
    ch_lo_aa = nc.declare_dram_parameter("ch_lo_aa", [RPC], fp32, isOutput=False)
    ch_hi_aa = nc.declare_dram_parameter("ch_hi_aa", [RPC], fp32, isOutput=False)
    ch_lo_sm = nc.declare_dram_parameter("ch_lo_sm", [RPC], fp32, isOutput=False)
    ch_hi_sm = nc.declare_dram_parameter("ch_hi_sm", [RPC], fp32, isOutput=False)
    aa_row = nc.declare_dram_parameter("aa_row", [RPC], fp32, isOutput=False)
    isaa_row = nc.declare_dram_parameter("isaa_row", [RPC], fp32, isOutput=False)
    Wp_d = nc.declare_dram_parameter("Wp", [32, PAIR], fp32, isOutput=False)
    W1_d = nc.declare_dram_parameter("W1", [PAIR, 2 * PAIR], fp32, isOutput=False)
    W2_d = nc.declare_dram_parameter("W2", [2 * PAIR, LOCAL], fp32, isOutput=False)
    Wl_d = nc.declare_dram_parameter("Wl", [288, PAIR], fp32, isOutput=False)
    ln1g_d = nc.declare_dram_parameter("ln1_g", [PAIR], fp32, isOutput=False)
    ln1b_d = nc.declare_dram_parameter("ln1_b", [PAIR], fp32, isOutput=False)
    ln2g_d = nc.declare_dram_parameter("ln2_g", [PAIR], fp32, isOutput=False)
    ln2b_d = nc.declare_dram_parameter("ln2_b", [PAIR], fp32, isOutput=False)
    b1_d = nc.declare_dram_parameter("b1", [2 * PAIR], fp32, isOutput=False)
    b2_d = nc.declare_dram_parameter("b2", [LOCAL], fp32, isOutput=False)
    io128_d = nc.declare_dram_parameter("iota128", [128], fp32, isOutput=False)
    io21_d = nc.declare_dram_parameter("iota21v", [32], fp32, isOutput=False)

    pair_out = nc.declare_dram_parameter(
        "pair_out", [PAIR, NBLK * 12 * 512], fp32, isOutput=True)
    local_out = nc.declare_dram_parameter("local_out", [PAIR, RPC], fp32, isOutput=True)
    nbr_out = nc.declare_dram_parameter("nbr_out", [RPC, KTOT], u32, isOutput=True)

    with TileContext(nc) as tc:
        with tc.tile_pool(name="setup", bufs=1) as setup, \
             tc.tile_pool(name="blk", bufs=2) as blk, \
             tc.tile_pool(name="vpool", bufs=2) as vpool, \
             tc.tile_pool(name="psA", bufs=3, space="PSUM") as psA, \
             tc.tile_pool(name="psS", bufs=2, space="PSUM") as psS, \
             tc.tile_pool(name="accps", bufs=1, space="PSUM") as accps:

            # ---------- constants ----------
            ones_r = setup.tile([1, 128], fp32, tag="ones_r")
            nc.vector.memset(ones_r, 1.0)
            ident = setup.tile([128, 128], fp32, tag="ident")
            iota_col_f = setup.tile([128, 1], fp32, tag="iocf")
            iota_row_f = setup.tile([128, 128], fp32, tag="iorf")
            nc.sync.dma_start(out=iota_col_f, in_=io128_d[:].rearrange("(p o) -> p o", o=1))
            nc.sync.dma_start(out=iota_row_f[0:1, :], in_=io128_d[:].rearrange("(o n) -> o n", o=1))
            iops = psA.tile([128, 512], fp32, tag="ps")
            nc.tensor.matmul(iops[:, 0:128], ones_r, iota_row_f[0:1, :], start=True, stop=True)
            nc.scalar.copy(iota_row_f, iops[:, 0:128])
            nc.vector.tensor_scalar(ident, iota_row_f, iota_col_f, None, OP.is_equal)


            ones_k = setup.tile([128, 1], fp32, tag="ones_k")       # reduce over 128 parts
            nc.vector.memset(ones_k, 1.0)
            ones_kb = setup.tile([128, 128], bf16, tag="ones_kb")   # replicate-sum lhsT
            nc.vector.memset(ones_kb, 1.0)
            epsT = setup.tile([128, 1], fp32, tag="epsT")
            nc.vector.memset(epsT, EPS)


            # ---------- weights / vectors ----------
            Wp4 = setup.tile([128, 4, PAIR], fp32, tag="Wp4")
            nc.vector.memset(Wp4, 0.0)
            nc.sync.dma_start(out=Wp4[0:32, 0, :], in_=Wp_d[:, :])
            W1 = setup.tile([128, 2 * PAIR], fp32, tag="W1")
            nc.sync.dma_start(out=W1, in_=W1_d[:, :])
            W2 = setup.tile([128, 2, LOCAL], fp32, tag="W2")
            nc.sync.dma_start(out=W2, in_=W2_d[:, :].rearrange("(t p) n -> p t n", p=128))
            Wl = setup.tile([128, 3, PAIR], fp32, tag="Wl")
            nc.sync.dma_start(out=Wl[:, 0, :], in_=Wl_d[0:128, :])
            nc.sync.dma_start(out=Wl[:, 1, :], in_=Wl_d[128:256, :])
            nc.sync.dma_start(out=Wl[0:32, 2, :], in_=Wl_d[256:288, :])
            Wp4b = setup.tile([128, 4, PAIR], bf16, tag="Wp4b")
            W1b = setup.tile([128, 2 * PAIR], bf16, tag="W1b")
            W2b = setup.tile([128, 2, LOCAL], bf16, tag="W2b")
            Wlb = setup.tile([128, 3, PAIR], bf16, tag="Wlb")
            nc.vector.tensor_copy(W1b, W1)
            nc.vector.tensor_copy(W2b, W2)

            ln1g = setup.tile([128, 1], fp32, tag="ln1g")
            ln1b = setup.tile([128, 1], fp32, tag="ln1b")
            ln2g = setup.tile([128, 1], fp32, tag="ln2g")
            ln2b = setup.tile([128, 1], fp32, tag="ln2b")
            nc.sync.dma_start(out=ln1g, in_=ln1g_d[:].rearrange("(p o) -> p o", o=1))
            nc.sync.dma_start(out=ln1b, in_=ln1b_d[:].rearrange("(p o) -> p o", o=1))
            nc.sync.dma_start(out=ln2g, in_=ln2g_d[:].rearrange("(p o) -> p o", o=1))
            nc.sync.dma_start(out=ln2b, in_=ln2b_d[:].rearrange("(p o) -> p o", o=1))
            b1 = setup.tile([128, 2], fp32, tag="b1")
            nc.sync.dma_start(out=b1, in_=b1_d[:].rearrange("(t p) -> p t", p=128))
            b2x = setup.tile([128, 2], fp32, tag="b2x")
            nc.sync.dma_start(out=b2x, in_=b2_d[:].rearrange("(t p) -> p t", p=128))
            nc.vector.tensor_scalar(b2x, b2x, float(KTOT), None, OP.mult)

            # center Wp / Wl columns (free axis) so LN mean-subtract is free
            wpm = setup.tile([32, 1], fp32, tag="wpm")
            nc.vector.tensor_reduce(wpm, Wp4[0:32, 0, :], mybir.AxisListType.X, OP.add)
            nc.vector.tensor_scalar(wpm, wpm, 1.0 / PAIR, None, OP.mult)
            nc.vector.tensor_scalar(Wp4[0:32, 0, :], Wp4[0:32, 0, :], wpm, None, OP.subtract)
            for q in range(1, 4):
                nc.sync.dma_start(out=Wp4[q * 32:(q + 1) * 32, q, :], in_=Wp4[0:32, 0, :])
            nc.vector.tensor_copy(Wp4b, Wp4)
            for t in range(3):
                pr = 128 if t < 2 else 32
                wlm = setup.tile([128, 1], fp32, tag="wlm")
                nc.vector.tensor_reduce(wlm[:pr], Wl[:pr, t, :], mybir.AxisListType.X, OP.add)
                nc.vector.tensor_scalar(wlm[:pr], wlm[:pr], 1.0 / PAIR, None, OP.mult)
                nc.vector.tensor_scalar(Wl[:pr, t, :], Wl[:pr, t, :], wlm[:pr], None, OP.subtract)
            nc.vector.tensor_copy(Wlb, Wl)

            # extras table [32, RPC]: row0 = is_aa, rows 1..21 = one_hot(aa)
            ex_all = setup.tile([32, RPC], fp32, tag="ex_all")
            nc.vector.memset(ex_all, 0.0)
            aarow_sb = setup.tile([1, RPC], fp32, tag="aarow_sb")
            nc.sync.dma_start(out=aarow_sb, in_=aa_row[:].rearrange("(o n) -> o n", o=1))
            for t0 in range(0, RPC, 512):
                w = min(512, RPC - t0)
                sl = slice(t0, t0 + w)
                bps = psA.tile([128, 512], fp32, tag="ps")
                nc.tensor.matmul(bps[0:21, 0:w], ones_r[:, 0:21], aarow_sb[:, sl],
                                 start=True, stop=True)
                nc.scalar.copy(ex_all[0:21, sl], bps[0:21, 0:w])
            nc.sync.dma_start(out=ex_all[21:22, :], in_=isaa_row[:].rearrange("(o n) -> o n", o=1))
            iota21 = setup.tile([32, 1], fp32, tag="io21")
            nc.sync.dma_start(out=iota21, in_=io21_d[:].rearrange("(p o) -> p o", o=1))
            nc.vector.tensor_scalar(ex_all[0:21, :], ex_all[0:21, :], iota21[0:21], None, OP.is_equal)
            ex_allb = setup.tile([32, RPC], bf16, tag="ex_allb")
            nc.vector.tensor_copy(ex_allb, ex_all)

            # per-row scalars, laid out [128, NBLK]
            rowsc = {}
            for nm, dv in (("own", own_col), ("laa", ch_lo_aa), ("haa", ch_hi_aa),
                           ("lsm", ch_lo_sm), ("hsm", ch_hi_sm)):
                t_ = setup.tile([128, NBLK], fp32, tag="rs_" + nm)
                nc.sync.dma_start(out=t_, in_=dv[:].rearrange("(b p) -> p b", p=128))
                rowsc[nm] = t_

            # ---------- rhs table [5, NT]: 2-> posT, ones, sq ----------
            rhs = setup.tile([5, NT], fp32, tag="rhs")
            nc.sync.dma_start(out=rhs, in_=posT[:, :])
            possq = vpool.tile([128, NT], fp32, tag="V")   # reuse V storage
            nc.vector.tensor_mul(possq[0:3, :], rhs[0:3, :], rhs[0:3, :])
            ones3 = setup.tile([3, 1], fp32, tag="ones3")
            nc.vector.memset(ones3, 1.0)
            sqtab_t = vpool.tile([128, NT], fp32, tag="V")
            sqtab = sqtab_t[0:1, :]
            for t in range(NT_TILES):
                sl = slice(t * 512, (t + 1) * 512)
                sq_ps = psA.tile([128, 512], fp32, tag="ps")
                nc.tensor.matmul(sq_ps[0:1, :], ones3, possq[0:3, sl], start=True, stop=True)
                nc.scalar.copy(sqtab[:, sl], sq_ps[0:1, :])
            nc.sync.dma_start(out=rhs[4:5, :], in_=sqtab)

            # ---------- lhsT [5, RPC]: 2*pos, -sq_i, -1 ----------
            lhsT = setup.tile([5, RPC], fp32, tag="lhsT")
            nc.sync.dma_start(out=lhsT, in_=pos_rows[:, :])
            nc.vector.tensor_scalar(lhsT[0:3, :], lhsT[0:3, :], 2.0, None, OP.mult)
            rsq = setup.tile([3, RPC], fp32, tag="rsq")
            nc.sync.dma_start(out=rsq, in_=pos_rows[0:3, :])
            nc.vector.tensor_mul(rsq, rsq, rsq)
            negones3 = setup.tile([3, 1], fp32, tag="negones3")
            nc.vector.memset(negones3, -1.0)
            sqi_tmp = setup.tile([1, RPC], fp32, tag="sqi_tmp")
            for t0 in range(0, RPC, 512):
                w = min(512, RPC - t0)
                sl = slice(t0, t0 + w)
                sq_ps2 = psA.tile([128, 512], fp32, tag="ps")
                nc.tensor.matmul(sq_ps2[0:1, 0:w], negones3, rsq[:, sl], start=True, stop=True)
                nc.scalar.copy(sqi_tmp[:, sl], sq_ps2[0:1, 0:w])
            # move row (partition 0) to partition 3 of lhsT via sbuf-sbuf DMA
            nc.sync.dma_start(out=lhsT[3:4, :], in_=sqi_tmp)

            # =====================================================
            # per-block processing
            # =====================================================
            for b in range(NBLK):
                lb = lhsT[:, b * BLK:(b + 1) * BLK]

                # ---- -d2 + evacuation ----
                V = vpool.tile([128, NT], fp32, tag="V")
                for t in range(NT_TILES):
                    sl = slice(t * 512, (t + 1) * 512)
                    ps = psA.tile([128, 512], fp32, tag="ps")
                    nc.tensor.matmul(ps, lb, rhs[:, sl], start=True, stop=True)
                    nc.scalar.copy(V[:, sl], ps)

                if stage < 2:
                    continue
                # ---- seg top8 ----
                S = blk.tile([128, NSEG * 8], fp32, tag="S")
                for s in range(NSEG):
                    nc.vector.max(S[:, s * 8:(s + 1) * 8], V[:, s * 128:(s + 1) * 128])

                # ---- merge ladders ----
                Wv = blk.tile([128, KTOT], fp32, tag="Wv")
                Gu = blk.tile([128, KTOT], u32, tag="Gu")
                Saa = S[:, :SEG_AA * 8]
                Ssm = S[:, SEG_AA * 8:NSEG * 8]
                Vaa = V[:, :NAAP]
                Vsm = V[:, NAAP:NT]
                for r in range(R_AA):
                    w8 = Wv[:, r * 8:(r + 1) * 8]
                    nc.vector.max(w8, Saa)
                    nc.vector.max_index(Gu[:, r * 8:(r + 1) * 8], w8, Vaa)
                    if r + 1 < R_AA:
                        nc.vector.match_replace(Saa, w8, Saa, NEG_BIG)
                for r in range(R_SM):
                    w8 = Wv[:, K_AA + r * 8:K_AA + (r + 1) * 8]
                    nc.vector.max(w8, Ssm)
                    nc.vector.max_index(Gu[:, K_AA + r * 8:K_AA + (r + 1) * 8], w8, Vsm)
                    if r + 1 < R_SM:
                        nc.vector.match_replace(Ssm, w8, Ssm, NEG_BIG)

                nc.sync.dma_start(out=nbr_out[b * BLK:(b + 1) * BLK, :], in_=Gu)

                if stage < 3:
                    continue
                # ---- features [128, 48] ----
                Gf = blk.tile([128, KTOT], fp32, tag="Gf")
                nc.vector.tensor_copy(Gf, Gu)
                nc.vector.tensor_scalar(Gf[:, K_AA:], Gf[:, K_AA:], float(NAAP), None, OP.add)

                d2p = blk.tile([128, KTOT], fp32, tag="d2p")
                nc.vector.tensor_scalar(d2p, Wv, -1.0, 1e-12, OP.mult, OP.max)
                dd = blk.tile([128, KTOT], fp32, tag="dd")
                nc.scalar.activation(dd, d2p, AF.Sqrt)

                sc = blk.tile([128, KTOT], fp32, tag="sc")
                m2 = blk.tile([128, KTOT], fp32, tag="m2")
                bsl = slice(b, b + 1)
                nc.vector.tensor_scalar(sc[:, :K_AA], Gf[:, :K_AA], rowsc["laa"][:, bsl], None, OP.is_ge)
                nc.vector.tensor_scalar(m2[:, :K_AA], Gf[:, :K_AA], rowsc["haa"][:, bsl], None, OP.is_lt)
                nc.vector.tensor_scalar(sc[:, K_AA:], Gf[:, K_AA:], rowsc["lsm"][:, bsl], None, OP.is_ge)
                nc.vector.tensor_scalar(m2[:, K_AA:], Gf[:, K_AA:], rowsc["hsm"][:, bsl], None, OP.is_lt)
                nc.vector.tensor_mul(sc, sc, m2)
                oc = blk.tile([128, KTOT], fp32, tag="oc")
                nc.vector.tensor_scalar(oc, sc, -1.0, 1.0, OP.mult, OP.add)
                sr = blk.tile([128, KTOT], fp32, tag="sr")
                nc.vector.tensor_scalar(sr, Gf, rowsc["own"][:, bsl], None, OP.is_equal)

                # rbf: scratch r-major [16 x 48], squared, exp into F strided
                F = blk.tile([128, KTOT * 32], fp32, tag="F")
                nc.vector.memset(F, 0.0)
                Rt = blk.tile([128, RBF_BINS * KTOT], fp32, tag="Rt")
                sig = (22.0 - 2.0) / RBF_BINS
                for r_ in range(RBF_BINS):
                    c_r = 2.0 + r_ * (20.0 / (RBF_BINS - 1))
                    nc.vector.tensor_scalar(
                        Rt[:, r_ * KTOT:(r_ + 1) * KTOT], dd, -c_r, None, OP.add)
                nc.vector.tensor_mul(Rt, Rt, Rt)
                Fv = F[:, :].rearrange("p (k f) -> p k f", f=32)
                Rv = Rt[:, :].rearrange("p (f k) -> p k f", k=KTOT)
                nc.scalar.activation(Fv[:, :, 0:RBF_BINS], Rv, AF.Exp,
                                     scale=-1.0 / (sig * sig))
                nc.vector.memset(Fv[:, 0:K_AA, 16:17], 1.0)
                nc.vector.tensor_copy(Fv[:, :, 17:18], sr[:, :].rearrange("p (k o) -> p k o", o=1))
                nc.vector.tensor_copy(Fv[:, :, 18:19], oc[:, :].rearrange("p (k o) -> p k o", o=1))

                if stage < 35:
                    continue
                # ---- transpose F -> FT [128, 12, 128] ----
                FT = blk.tile([128, 12, 128], bf16, tag="FT")
                for j in range(12):
                    tps = psA.tile([128, 512], fp32, tag="ps")
                    nc.tensor.transpose(tps[:, 0:128], F[:, j * 128:(j + 1) * 128], ident)
                    nc.scalar.copy(FT[:, j, :], tps[:, 0:128])

                if stage < 40:
                    continue
                # ---- MLP over 12 j-tiles ----
                mlp_ps0 = accps.tile([128, 512], fp32, tag="mlpa")
                mlp_ps1 = accps.tile([128, 512], fp32, tag="mlpb")
                for j in range(12):
                    pairps = psA.tile([128, 512], fp32, tag="ps")
                    for kk in range(4):
                        nc.tensor.matmul(
                            pairps[:, kk * 128:(kk + 1) * 128],
                            Wp4b[:, kk, :],
                            FT[:, j, :], start=True, stop=True)
                    # LN1
                    sqt = blk.tile([128, 512], fp32, tag="sqt")
                    nc.scalar.activation(sqt, pairps, AF.Square)
                    ssq = psS.tile([1, 512], fp32, tag="small")
                    nc.tensor.matmul(ssq, ones_k, sqt, start=True, stop=True)
                    sgt = blk.tile([1, 512], fp32, tag="sgt")
                    nc.scalar.activation(sgt, ssq, AF.Sqrt, scale=1.0 / PAIR, bias=epsT[0:1, :])
                    rsg = blk.tile([1, 512], fp32, tag="rsg")
                    nc.vector.reciprocal(rsg, sgt)
                    rep = blk.tile([128, 512], fp32, tag="rep")
                    nc.gpsimd.partition_broadcast(rep, rsg)
                    pair_sb = blk.tile([128, 512], fp32, tag="pair_sb")
                    nc.vector.scalar_tensor_tensor(
                        pair_sb, pairps, ln1g, rep, OP.mult, OP.mult)
                    nc.vector.tensor_scalar(pair_sb, pair_sb, ln1b, None, OP.add)
                    nc.sync.dma_start(
                        out=pair_out[:, (b * 12 + j) * 512:(b * 12 + j + 1) * 512],
                        in_=pair_sb)
                    # W1 + gelu
                    hs = blk.tile([128, 2, 512], fp32, tag="hs")
                    for t in range(2):
                        hps = psA.tile([128, 512], fp32, tag="ps")
                        nc.tensor.matmul(hps, W1[:, t * 128:(t + 1) * 128], pair_sb,
                                         start=True, stop=True)
                        gf = AF.Gelu_apprx_tanh if use_gelu else AF.Square
                        nc.scalar.activation(hs[:, t, :], hps, gf,
                                             bias=b1[:, t:t + 1])
                    # W2 accumulate over j
                    for t, mps in ((0, mlp_ps0), (1, mlp_ps1)):
                        for s in range(2):
                            nc.tensor.matmul(
                                mps, W2[:, s, t * 128:(t + 1) * 128], hs[:, s, :],
                                start=(j == 0 and s == 0), stop=(j == 11 and s == 1),
                                skip_group_check=True)

                if stage < 50:
                    continue
                # ---- reduce over kk; + 48*b2 ----
                pw = blk.tile([128, 2, 128], bf16, tag="pw")
                with nc.allow_low_precision(reason="k-sum of 4 into bf16 for Wl rhs"):
                    for t, mps in ((0, mlp_ps0), (1, mlp_ps1)):
                        nc.vector.tensor_reduce(
                            pw[:, t, :], mps[:, :].rearrange("p (k i) -> p i k", k=4),
                            mybir.AxisListType.X, OP.add)
                        nc.vector.tensor_scalar(pw[:, t, :], pw[:, t, :], b2x[:, t:t + 1], None, OP.add)

                # ---- Wl matmuls + LN2 ----
                rsl = slice(b * BLK, (b + 1) * BLK)
                y2 = psS.tile([128, 128], fp32, tag="small")
                nc.tensor.matmul(y2, Wlb[:, 0, :], pw[:, 0, :], start=True, stop=False)
                nc.tensor.matmul(y2, Wlb[:, 1, :], pw[:, 1, :], start=False, stop=False)
                nc.tensor.matmul(y2, Wlb[0:32, 2, :], ex_allb[:, rsl], start=False, stop=True)
                sq2 = blk.tile([128, 128], bf16, tag="sq2")
                nc.scalar.activation(sq2, y2, AF.Square)
                srep2 = psS.tile([128, 512], fp32, tag="small")
                nc.tensor.matmul(srep2[:, 0:128], ones_kb, sq2, start=True, stop=True)
                rs2 = blk.tile([128, 128], bf16, tag="rs2")
                nc.scalar.activation(rs2, srep2[:, 0:128], AF.Abs_reciprocal_sqrt,
                                     scale=1.0 / PAIR, bias=epsT)
                loc = blk.tile([128, 128], fp32, tag="loc")
                nc.vector.scalar_tensor_tensor(loc, y2, ln2g, rs2, OP.mult, OP.mult)
                nc.vector.tensor_scalar(loc, loc, ln2b, None, OP.add)
                nc.sync.dma_start(out=local_out[:, rsl], in_=loc)

    nc.finalize()
    RPC, NBLK = RPC_SAVE, NBLK_SAVE
    return nc


def _host_prep(inputs):
    pos = np.asarray(inputs["positions"])[:, 1].astype(np.float32)      # [N,3]
    is_aa = np.asarray(inputs["is_aa_int"]).astype(bool)
    chain = np.asarray(inputs["chain_index"]).astype(np.int64)

    aa_idx = np.where(is_aa)[0]
    sm_idx = np.where(~is_aa)[0]
    n_aa, n_sm = len(aa_idx), len(sm_idx)
    NAAP = _ceil_to(n_aa, 512)
    NSMP = _ceil_to(n_sm, 512)
    NT = NAAP + NSMP

    perm = np.concatenate([aa_idx, sm_idx])            # row r -> orig atom
    col2orig = np.full(NT, 0, np.int64)
    col2orig[:n_aa] = aa_idx
    col2orig[NAAP:NAAP + n_sm] = sm_idx

    table = np.full((NT, 3), 1.0e4, np.float32)
    table[:n_aa] = pos[aa_idx]
    table[NAAP:NAAP + n_sm] = pos[sm_idx]
    npad_a = NAAP - n_aa
    table[n_aa:NAAP, 0] += 64.0 * np.arange(1, npad_a + 1, dtype=np.float32)
    npad_s = NSMP - n_sm
    table[NAAP + n_sm:, 1] += 64.0 * np.arange(1, npad_s + 1, dtype=np.float32)

    row_col = np.empty(N, np.float32)                  # row -> its table col
    row_col[:n_aa] = np.arange(n_aa)
    row_col[n_aa:] = NAAP + np.arange(n_sm)

    ch_of_row = chain[perm]
    ch_aa = chain[aa_idx]; ch_sm = chain[sm_idx]
    nch = int(chain.max()) + 1
    aa_st = np.searchsorted(ch_aa, np.arange(nch + 1))
    sm_st = np.searchsorted(ch_sm, np.arange(nch + 1))
    c = ch_of_row
    lo_aa = aa_st[c].astype(np.float32); hi_aa = aa_st[c + 1].astype(np.float32)
    lo_sm = (NAAP + sm_st[c]).astype(np.float32); hi_sm = (NAAP + sm_st[c + 1]).astype(np.float32)

    return dict(NT=NT, NAAP=NAAP, perm=perm, col2orig=col2orig, table=table,
                row_col=row_col, lo_aa=lo_aa, hi_aa=hi_aa, lo_sm=lo_sm, hi_sm=hi_sm)


def kernel(_trace=False, _trace_dir=None, **inputs):
    import concourse.bass_utils as bass_utils

    hp = _host_prep(inputs)
    NT, NAAP = hp["NT"], hp["NAAP"]
    perm = hp["perm"]

    key = (NT, NAAP)
    if key not in _CACHE:
        _CACHE[key] = _build_graph(NT, NAAP)
    nc = _CACHE[key]

    NTv = hp["NT"]
    posT = np.concatenate([
        np.ascontiguousarray(hp["table"].T),
        np.ones((1, NTv), np.float32),
        np.zeros((1, NTv), np.float32)], axis=0)               # [5, NT]
    pos = np.asarray(inputs["positions"])[:, 1].astype(np.float32)
    aa = np.asarray(inputs["aa"]).astype(np.float32)
    isaa_f = np.asarray(inputs["is_aa_int"]).astype(np.float32)

    Wp_p = np.zeros((32, PAIR), np.float32)
    Wp_p[:19] = np.asarray(inputs["Wp"], np.float32)
    Wl_full = np.asarray(inputs["Wl"], np.float32)
    Wl_p = np.zeros((288, PAIR), np.float32)
    Wl_p[:256] = Wl_full[:256]
    Wl_p[256:277] = Wl_full[257:278]      # one_hot rows first
    Wl_p[277] = Wl_full[256]              # then is_aa row

    in_maps = []
    for c in range(NCORES):
        rs = slice(c * RPC, (c + 1) * RPC)
        rows = perm[rs]
        in_maps.append({
            "pos_rows": np.concatenate([
                np.ascontiguousarray(pos[rows].T),
                np.zeros((1, RPC), np.float32),
                np.full((1, RPC), -1.0, np.float32)], axis=0),
            "posT": posT,
            "own_col": hp["row_col"][rs],
            "ch_lo_aa": hp["lo_aa"][rs], "ch_hi_aa": hp["hi_aa"][rs],
            "ch_lo_sm": hp["lo_sm"][rs], "ch_hi_sm": hp["hi_sm"][rs],
            "aa_row": aa[rows], "isaa_row": isaa_f[rows],
            "Wp": Wp_p, "W1": np.asarray(inputs["W1"], np.float32),
            "W2": np.asarray(inputs["W2"], np.float32), "Wl": Wl_p,
            "ln1_g": np.asarray(inputs["ln1_g"], np.float32),
            "ln1_b": np.asarray(inputs["ln1_b"], np.float32),
            "ln2_g": np.asarray(inputs["ln2_g"], np.float32),
            "ln2_b": np.asarray(inputs["ln2_b"], np.float32),
            "b1": np.asarray(inputs["b1"], np.float32),
            "b2": np.asarray(inputs["b2"], np.float32),
            "iota128": np.arange(128, dtype=np.float32),
            "iota21v": np.concatenate([np.arange(21), np.zeros(11)]).astype(np.float32),
        })

    kw = {}
    if _trace:
        kw = dict(trace=True, tmpdir=_trace_dir)
    res = bass_utils.run_bass_kernel_spmd(nc, in_maps, core_ids=list(range(NCORES)), **kw)
    if _trace:
        print("HW exec time:", res.exec_time_ns, "ns")
    results = res.results

    # ---- host unshard / unpermute ----
    local = np.empty((N, PAIR), np.float32)
    pair = np.empty((N, KTOT, PAIR), np.float32)
    nbrs = np.empty((N, KTOT), np.int32)

    for c in range(NCORES):
        r = results[c]
        rows = perm[c * RPC:(c + 1) * RPC]
        local[rows] = np.asarray(r["local_out"]).T
        po = np.asarray(r["pair_out"]).reshape(PAIR, NBLK, 12, 4, 128)
        po = po.transpose(1, 4, 2, 3, 0).reshape(RPC, KTOT, PAIR)
        pair[rows] = po
        nb = np.asarray(r["nbr_out"]).astype(np.int64)
        nb[:, K_AA:] += NAAP
        nbrs[rows] = hp["col2orig"][nb]

    mask = np.asarray(inputs["all_atom_mask"])[:, 1] > 0
    return (local, pair, nbrs.astype(np.int32), np.asarray(mask))


# revision 28
# speedup vs baseline: 1.6727x; 1.5898x over previous
"""AtomPairEmbedding Trainium2 kernel — 8-core SPMD, row-parallel.

Strategy (hardcoded for N=8192, K=32+16, PAIR=128, LOCAL=256):
 - Host glue: stable-permute atoms so aa atoms occupy table cols [0, n_aa_pad)
   and small molecules [n_aa_pad, NT); pad each category with far-away
   sentinels to a multiple of 512. Shard rows (permuted order) 1024/core.
 - Device per 128-row block:
     * PE computes  -d2 = 2*dot - sq_i - sq_j  via a K=5 matmul over the
       full table (NT/512 tiles) into PSUM; ScalarE evacuates to SBUF (V).
     * VectorE: per-128-col-segment max8 (top-8 per segment), then a
       max8/match_replace ladder on the segment candidates per category,
       with max_index against V to recover global column ids.
     * Edge features (16 RBF + type + same_residue + other_chain) built with
       tiny [128,48] ops; no gathers: type/chain/residue features derive
       from column ranges (category-contiguous layout + sorted chains).
     * Feature tile transposed via PE, then feature-on-partition MLP:
       pair = LN(edge @ Wp); h = gelu(pair@W1+b1); mlp=h@W2 accumulated in
       PSUM over all 48 neighbours; local = LN(local_in @ Wl).
 - Host glue out: un-permute rows, map table cols back to original atom ids.
"""

import numpy as np

N = 8192
K_AA, K_SMOL = 32, 16
KTOT = K_AA + K_SMOL
PAIR = 128
LOCAL = 256
RBF_BINS = 16
EPS = 1e-5
NCORES = 8
RPC = N // NCORES          # rows per core
BLK = 128                  # rows per block
NBLK = RPC // BLK
NEG_BIG = -3.0e38

_CACHE = {}


def _ceil_to(x, m):
    return (x + m - 1) // m * m


def _build_graph(NT, NAAP, nblk=NBLK, rpc=RPC, use_gelu=True, stage=50):
    """Build the single-core Bass graph (SPMD across 8 cores)."""
    import concourse.bacc as bacc
    import concourse.mybir as mybir
    from concourse.tile import TileContext

    fp32 = mybir.dt.float32
    bf16 = mybir.dt.bfloat16
    u32 = mybir.dt.uint32
    i32 = mybir.dt.int32
    AF = mybir.ActivationFunctionType
    OP = mybir.AluOpType

    global RPC, NBLK
    RPC_SAVE, NBLK_SAVE = RPC, NBLK
    RPC, NBLK = rpc, nblk
    NSEG = NT // 128               # segments of 128
    SEG_AA = NAAP // 128
    NT_TILES = NT // 512
    R_AA = K_AA // 8               # merge rounds aa
    R_SM = K_SMOL // 8

    nc = bacc.Bacc()

    # ---- DRAM parameters (per-core values via in_maps) ----
    pos_rows = nc.declare_dram_parameter("pos_rows", [5, RPC], fp32, isOutput=False)
    posT = nc.declare_dram_parameter("posT", [5, NT], fp32, isOutput=False)
    own_col = nc.declare_dram_parameter("own_col", [RPC], fp32, isOutput=False)
    ch_lo_aa = nc.declare_dram_parameter("ch_lo_aa", [RPC], fp32, isOutput=False)
    ch_hi_aa = nc.declare_dram_parameter("ch_hi_aa", [RPC], fp32, isOutput=False)
    ch_lo_sm = nc.declare_dram_parameter("ch_lo_sm", [RPC], fp32, isOutput=False)
    ch_hi_sm = nc.declare_dram_parameter("ch_hi_sm", [RPC], fp32, isOutput=False)
    aa_row = nc.declare_dram_parameter("aa_row", [RPC], fp32, isOutput=False)
    isaa_row = nc.declare_dram_parameter("isaa_row", [RPC], fp32, isOutput=False)
    Wp_d = nc.declare_dram_parameter("Wp", [32, PAIR], fp32, isOutput=False)
    W1_d = nc.declare_dram_parameter("W1", [PAIR, 2 * PAIR], fp32, isOutput=False)
    W2_d = nc.declare_dram_parameter("W2", [2 * PAIR, LOCAL], fp32, isOutput=False)
    Wl_d = nc.declare_dram_parameter("Wl", [288, PAIR], fp32, isOutput=False)
    ln1g_d = nc.declare_dram_parameter("ln1_g", [PAIR], fp32, isOutput=False)
    ln1b_d = nc.declare_dram_parameter("ln1_b", [PAIR], fp32, isOutput=False)
    ln2g_d = nc.declare_dram_parameter("ln2_g", [PAIR], fp32, isOutput=False)
    ln2b_d = nc.declare_dram_parameter("ln2_b", [PAIR], fp32, isOutput=False)
    b1_d = nc.declare_dram_parameter("b1", [2 * PAIR], fp32, isOutput=False)
    b2_d = nc.declare_dram_parameter("b2", [LOCAL], fp32, isOutput=False)
    io128_d = nc.declare_dram_parameter("iota128", [128], fp32, isOutput=False)
    io21_d = nc.declare_dram_parameter("iota21v", [32], fp32, isOutput=False)

    pair_out = nc.declare_dram_parameter(
        "pair_out", [PAIR, NBLK * 12 * 512], fp32, isOutput=True)
    local_out = nc.declare_dram_parameter("local_out", [PAIR, RPC], fp32, isOutput=True)
    nbr_out = nc.declare_dram_parameter("nbr_out", [RPC, KTOT], u32, isOutput=True)

    with TileContext(nc) as tc:
        with tc.tile_pool(name="setup", bufs=1) as setup, \
             tc.tile_pool(name="blk", bufs=2) as blk, \
             tc.tile_pool(name="vpool", bufs=2) as vpool, \
             tc.tile_pool(name="psA", bufs=3, space="PSUM") as psA, \
             tc.tile_pool(name="psS", bufs=2, space="PSUM") as psS, \
             tc.tile_pool(name="accps", bufs=1, space="PSUM") as accps:

            # ---------- constants ----------
            ones_r = setup.tile([1, 128], fp32, tag="ones_r")
            nc.vector.memset(ones_r, 1.0)
            ident = setup.tile([128, 128], fp32, tag="ident")
            iota_col_f = setup.tile([128, 1], fp32, tag="iocf")
            iota_row_f = setup.tile([128, 128], fp32, tag="iorf")
            nc.sync.dma_start(out=iota_col_f, in_=io128_d[:].rearrange("(p o) -> p o", o=1))
            nc.sync.dma_start(out=iota_row_f[0:1, :], in_=io128_d[:].rearrange("(o n) -> o n", o=1))
            iops = psA.tile([128, 512], fp32, tag="ps")
            nc.tensor.matmul(iops[:, 0:128], ones_r, iota_row_f[0:1, :], start=True, stop=True)
            nc.scalar.copy(iota_row_f, iops[:, 0:128])
            nc.vector.tensor_scalar(ident, iota_row_f, iota_col_f, None, OP.is_equal)


            ones_k = setup.tile([128, 1], fp32, tag="ones_k")       # reduce over 128 parts
            nc.vector.memset(ones_k, 1.0)
            ones_kb = setup.tile([128, 128], bf16, tag="ones_kb")   # replicate-sum lhsT
            nc.vector.memset(ones_kb, 1.0)
            epsT = setup.tile([128, 1], fp32, tag="epsT")
            nc.vector.memset(epsT, EPS)


            # ---------- weights / vectors ----------
            Wp4 = setup.tile([128, 4, PAIR], fp32, tag="Wp4")
            nc.vector.memset(Wp4, 0.0)
            nc.sync.dma_start(out=Wp4[0:32, 0, :], in_=Wp_d[:, :])
            W1 = setup.tile([128, 2 * PAIR], fp32, tag="W1")
            nc.sync.dma_start(out=W1, in_=W1_d[:, :])
            W2 = setup.tile([128, 2, LOCAL], fp32, tag="W2")
            nc.sync.dma_start(out=W2, in_=W2_d[:, :].rearrange("(t p) n -> p t n", p=128))
            Wl = setup.tile([128, 3, PAIR], fp32, tag="Wl")
            nc.sync.dma_start(out=Wl[:, 0, :], in_=Wl_d[0:128, :])
            nc.sync.dma_start(out=Wl[:, 1, :], in_=Wl_d[128:256, :])
            nc.sync.dma_start(out=Wl[0:32, 2, :], in_=Wl_d[256:288, :])
            Wp4b = setup.tile([128, 4, PAIR], bf16, tag="Wp4b")
            W1b = setup.tile([128, 2 * PAIR], bf16, tag="W1b")
            W2b = setup.tile([128, 2, LOCAL], bf16, tag="W2b")
            Wlb = setup.tile([128, 3, PAIR], bf16, tag="Wlb")
            nc.vector.tensor_copy(W1b, W1)
            nc.vector.tensor_copy(W2b, W2)

            ln1g = setup.tile([128, 1], fp32, tag="ln1g")
            ln1b = setup.tile([128, 1], fp32, tag="ln1b")
            ln2g = setup.tile([128, 1], fp32, tag="ln2g")
            ln2b = setup.tile([128, 1], fp32, tag="ln2b")
            nc.sync.dma_start(out=ln1g, in_=ln1g_d[:].rearrange("(p o) -> p o", o=1))
            nc.sync.dma_start(out=ln1b, in_=ln1b_d[:].rearrange("(p o) -> p o", o=1))
            nc.sync.dma_start(out=ln2g, in_=ln2g_d[:].rearrange("(p o) -> p o", o=1))
            nc.sync.dma_start(out=ln2b, in_=ln2b_d[:].rearrange("(p o) -> p o", o=1))
            b1 = setup.tile([128, 2], fp32, tag="b1")
            nc.sync.dma_start(out=b1, in_=b1_d[:].rearrange("(t p) -> p t", p=128))
            b2x = setup.tile([128, 2], fp32, tag="b2x")
            nc.sync.dma_start(out=b2x, in_=b2_d[:].rearrange("(t p) -> p t", p=128))
            nc.vector.tensor_scalar(b2x, b2x, float(KTOT), None, OP.mult)

            # center Wp / Wl columns (free axis) so LN mean-subtract is free
            wpm = setup.tile([32, 1], fp32, tag="wpm")
            nc.vector.tensor_reduce(wpm, Wp4[0:32, 0, :], mybir.AxisListType.X, OP.add)
            nc.vector.tensor_scalar(wpm, wpm, 1.0 / PAIR, None, OP.mult)
            nc.vector.tensor_scalar(Wp4[0:32, 0, :], Wp4[0:32, 0, :], wpm, None, OP.subtract)
            for q in range(1, 4):
                nc.sync.dma_start(out=Wp4[q * 32:(q + 1) * 32, q, :], in_=Wp4[0:32, 0, :])
            nc.vector.tensor_copy(Wp4b, Wp4)
            for t in range(3):
                pr = 128 if t < 2 else 32
                wlm = setup.tile([128, 1], fp32, tag="wlm")
                nc.vector.tensor_reduce(wlm[:pr], Wl[:pr, t, :], mybir.AxisListType.X, OP.add)
                nc.vector.tensor_scalar(wlm[:pr], wlm[:pr], 1.0 / PAIR, None, OP.mult)
                nc.vector.tensor_scalar(Wl[:pr, t, :], Wl[:pr, t, :], wlm[:pr], None, OP.subtract)
            nc.vector.tensor_copy(Wlb, Wl)

            # extras table [32, RPC]: row0 = is_aa, rows 1..21 = one_hot(aa)
            ex_all = setup.tile([32, RPC], fp32, tag="ex_all")
            nc.vector.memset(ex_all, 0.0)
            aarow_sb = setup.tile([1, RPC], fp32, tag="aarow_sb")
            nc.sync.dma_start(out=aarow_sb, in_=aa_row[:].rearrange("(o n) -> o n", o=1))
            for t0 in range(0, RPC, 512):
                w = min(512, RPC - t0)
                sl = slice(t0, t0 + w)
                bps = psA.tile([128, 512], fp32, tag="ps")
                nc.tensor.matmul(bps[0:21, 0:w], ones_r[:, 0:21], aarow_sb[:, sl],
                                 start=True, stop=True)
                nc.scalar.copy(ex_all[0:21, sl], bps[0:21, 0:w])
            nc.sync.dma_start(out=ex_all[21:22, :], in_=isaa_row[:].rearrange("(o n) -> o n", o=1))
            iota21 = setup.tile([32, 1], fp32, tag="io21")
            nc.sync.dma_start(out=iota21, in_=io21_d[:].rearrange("(p o) -> p o", o=1))
            nc.vector.tensor_scalar(ex_all[0:21, :], ex_all[0:21, :], iota21[0:21], None, OP.is_equal)
            ex_allb = setup.tile([32, RPC], bf16, tag="ex_allb")
            nc.vector.tensor_copy(ex_allb, ex_all)

            # per-row scalars, laid out [128, NBLK]
            rowsc = {}
            for nm, dv in (("own", own_col), ("laa", ch_lo_aa), ("haa", ch_hi_aa),
                           ("lsm", ch_lo_sm), ("hsm", ch_hi_sm)):
                t_ = setup.tile([128, NBLK], fp32, tag="rs_" + nm)
                nc.sync.dma_start(out=t_, in_=dv[:].rearrange("(b p) -> p b", p=128))
                rowsc[nm] = t_

            # ---------- rhs table [5, NT]: 2-> posT, ones, sq ----------
            rhs = setup.tile([5, NT], fp32, tag="rhs")
            nc.sync.dma_start(out=rhs, in_=posT[:, :])
            possq = vpool.tile([128, NT], fp32, tag="V")   # reuse V storage
            nc.vector.tensor_mul(possq[0:3, :], rhs[0:3, :], rhs[0:3, :])
            ones3 = setup.tile([3, 1], fp32, tag="ones3")
            nc.vector.memset(ones3, 1.0)
            sqtab_t = vpool.tile([128, NT], fp32, tag="V")
            sqtab = sqtab_t[0:1, :]
            for t in range(NT_TILES):
                sl = slice(t * 512, (t + 1) * 512)
                sq_ps = psA.tile([128, 512], fp32, tag="ps")
                nc.tensor.matmul(sq_ps[0:1, :], ones3, possq[0:3, sl], start=True, stop=True)
                nc.scalar.copy(sqtab[:, sl], sq_ps[0:1, :])
            nc.sync.dma_start(out=rhs[4:5, :], in_=sqtab)

            # ---------- lhsT [5, RPC]: 2*pos, -sq_i, -1 ----------
            lhsT = setup.tile([5, RPC], fp32, tag="lhsT")
            nc.sync.dma_start(out=lhsT, in_=pos_rows[:, :])
            nc.vector.tensor_scalar(lhsT[0:3, :], lhsT[0:3, :], 2.0, None, OP.mult)
            rsq = setup.tile([3, RPC], fp32, tag="rsq")
            nc.sync.dma_start(out=rsq, in_=pos_rows[0:3, :])
            nc.vector.tensor_mul(rsq, rsq, rsq)
            negones3 = setup.tile([3, 1], fp32, tag="negones3")
            nc.vector.memset(negones3, -1.0)
            sqi_tmp = setup.tile([1, RPC], fp32, tag="sqi_tmp")
            for t0 in range(0, RPC, 512):
                w = min(512, RPC - t0)
                sl = slice(t0, t0 + w)
                sq_ps2 = psA.tile([128, 512], fp32, tag="ps")
                nc.tensor.matmul(sq_ps2[0:1, 0:w], negones3, rsq[:, sl], start=True, stop=True)
                nc.scalar.copy(sqi_tmp[:, sl], sq_ps2[0:1, 0:w])
            # move row (partition 0) to partition 3 of lhsT via sbuf-sbuf DMA
            nc.sync.dma_start(out=lhsT[3:4, :], in_=sqi_tmp)

            # =====================================================
            # per-block processing
            # =====================================================
            for b in range(NBLK):
                lb = lhsT[:, b * BLK:(b + 1) * BLK]

                # ---- -d2 + evacuation ----
                V = vpool.tile([128, NT], fp32, tag="V")
                for t in range(NT_TILES):
                    sl = slice(t * 512, (t + 1) * 512)
                    ps = psA.tile([128, 512], fp32, tag="ps")
                    nc.tensor.matmul(ps, lb, rhs[:, sl], start=True, stop=True)
                    nc.scalar.copy(V[:, sl], ps)

                if stage < 2:
                    continue
                # ---- seg top8 ----
                S = blk.tile([128, NSEG * 8], fp32, tag="S")
                for s in range(NSEG):
                    nc.vector.max(S[:, s * 8:(s + 1) * 8], V[:, s * 128:(s + 1) * 128])

                # ---- merge ladders ----
                Wv = blk.tile([128, KTOT], fp32, tag="Wv")
                Gu = blk.tile([128, KTOT], u32, tag="Gu")
                Saa = S[:, :SEG_AA * 8]
                Ssm = S[:, SEG_AA * 8:NSEG * 8]
                Vaa = V[:, :NAAP]
                Vsm = V[:, NAAP:NT]
                for r in range(R_AA):
                    w8 = Wv[:, r * 8:(r + 1) * 8]
                    nc.vector.max(w8, Saa)
                    nc.vector.max_index(Gu[:, r * 8:(r + 1) * 8], w8, Vaa)
                    if r + 1 < R_AA:
                        nc.vector.match_replace(Saa, w8, Saa, NEG_BIG)
                for r in range(R_SM):
                    w8 = Wv[:, K_AA + r * 8:K_AA + (r + 1) * 8]
                    nc.vector.max(w8, Ssm)
                    nc.vector.max_index(Gu[:, K_AA + r * 8:K_AA + (r + 1) * 8], w8, Vsm)
                    if r + 1 < R_SM:
                        nc.vector.match_replace(Ssm, w8, Ssm, NEG_BIG)

                nc.sync.dma_start(out=nbr_out[b * BLK:(b + 1) * BLK, :], in_=Gu)

                if stage < 3:
                    continue
                # ---- features [128, 48] ----
                Gf = blk.tile([128, KTOT], fp32, tag="Gf")
                nc.vector.tensor_copy(Gf, Gu)
                nc.vector.tensor_scalar(Gf[:, K_AA:], Gf[:, K_AA:], float(NAAP), None, OP.add)

                d2p = blk.tile([128, KTOT], fp32, tag="d2p")
                nc.vector.tensor_scalar(d2p, Wv, -1.0, 1e-12, OP.mult, OP.max)
                dd = blk.tile([128, KTOT], fp32, tag="dd")
                nc.scalar.activation(dd, d2p, AF.Sqrt)

                sc = blk.tile([128, KTOT], fp32, tag="sc")
                m2 = blk.tile([128, KTOT], fp32, tag="m2")
                bsl = slice(b, b + 1)
                nc.vector.tensor_scalar(sc[:, :K_AA], Gf[:, :K_AA], rowsc["laa"][:, bsl], None, OP.is_ge)
                nc.vector.tensor_scalar(m2[:, :K_AA], Gf[:, :K_AA], rowsc["haa"][:, bsl], None, OP.is_lt)
                nc.vector.tensor_scalar(sc[:, K_AA:], Gf[:, K_AA:], rowsc["lsm"][:, bsl], None, OP.is_ge)
                nc.vector.tensor_scalar(m2[:, K_AA:], Gf[:, K_AA:], rowsc["hsm"][:, bsl], None, OP.is_lt)
                nc.vector.tensor_mul(sc, sc, m2)
                oc = blk.tile([128, KTOT], fp32, tag="oc")
                nc.vector.tensor_scalar(oc, sc, -1.0, 1.0, OP.mult, OP.add)
                sr = blk.tile([128, KTOT], fp32, tag="sr")
                nc.vector.tensor_scalar(sr, Gf, rowsc["own"][:, bsl], None, OP.is_equal)

                # rbf: scratch r-major [16 x 48], squared, exp into F strided
                F = blk.tile([128, KTOT * 32], fp32, tag="F")
                nc.vector.memset(F, 0.0)
                Rt = blk.tile([128, RBF_BINS * KTOT], fp32, tag="Rt")
                sig = (22.0 - 2.0) / RBF_BINS
                for r_ in range(RBF_BINS):
                    c_r = 2.0 + r_ * (20.0 / (RBF_BINS - 1))
                    nc.vector.tensor_scalar(
                        Rt[:, r_ * KTOT:(r_ + 1) * KTOT], dd, -c_r, None, OP.add)
                nc.vector.tensor_mul(Rt, Rt, Rt)
                Fv = F[:, :].rearrange("p (k f) -> p k f", f=32)
                Rv = Rt[:, :].rearrange("p (f k) -> p k f", k=KTOT)
                nc.scalar.activation(Fv[:, :, 0:RBF_BINS], Rv, AF.Exp,
                                     scale=-1.0 / (sig * sig))
                nc.vector.memset(Fv[:, 0:K_AA, 16:17], 1.0)
                nc.vector.tensor_copy(Fv[:, :, 17:18], sr[:, :].rearrange("p (k o) -> p k o", o=1))
                nc.vector.tensor_copy(Fv[:, :, 18:19], oc[:, :].rearrange("p (k o) -> p k o", o=1))

                if stage < 35:
                    continue
                # ---- transpose F -> FT [128, 12, 128] ----
                FT = blk.tile([128, 12, 128], bf16, tag="FT")
                for j in range(12):
                    tps = psA.tile([128, 512], fp32, tag="ps")
                    nc.tensor.transpose(tps[:, 0:128], F[:, j * 128:(j + 1) * 128], ident)
                    nc.scalar.copy(FT[:, j, :], tps[:, 0:128])

                if stage < 40:
                    continue
                # ---- MLP over 12 j-tiles ----
                mlp_ps0 = accps.tile([128, 512], fp32, tag="mlpa")
                mlp_ps1 = accps.tile([128, 512], fp32, tag="mlpb")
                for j in range(12):
                    pairps = psA.tile([128, 512], fp32, tag="ps")
                    for kk in range(4):
                        nc.tensor.matmul(
                            pairps[:, kk * 128:(kk + 1) * 128],
                            Wp4b[:, kk, :],
                            FT[:, j, :], start=True, stop=True)
                    # LN1: replicated sum-of-squares + abs_rsqrt (no table thrash ops)
                    sqt = blk.tile([128, 512], bf16, tag="sqt")
                    nc.scalar.activation(sqt, pairps, AF.Square)
                    srep = psS.tile([128, 512], fp32, tag="small")
                    nc.tensor.matmul(srep, ones_kb, sqt, start=True, stop=True)
                    rsig = blk.tile([128, 512], bf16, tag="rsg")
                    nc.scalar.activation(rsig, srep, AF.Abs_reciprocal_sqrt,
                                         scale=1.0 / PAIR, bias=epsT)
                    pair_b = blk.tile([128, 512], bf16, tag="pair_sb")
                    nc.vector.scalar_tensor_tensor(
                        pair_b, pairps, ln1g, rsig, OP.mult, OP.mult)
                    nc.vector.tensor_scalar(pair_b, pair_b, ln1b, None, OP.add)
                    pair_f = blk.tile([128, 512], fp32, tag="pair_f")
                    nc.vector.tensor_copy(pair_f, pair_b)
                    nc.sync.dma_start(
                        out=pair_out[:, (b * 12 + j) * 512:(b * 12 + j + 1) * 512],
                        in_=pair_f)
                    # W1 + gelu
                    hs = blk.tile([128, 2, 512], bf16, tag="hs")
                    for t in range(2):
                        hps = psA.tile([128, 512], fp32, tag="ps")
                        nc.tensor.matmul(hps, W1b[:, t * 128:(t + 1) * 128], pair_b,
                                         start=True, stop=True)
                        gf = AF.Gelu_apprx_tanh if use_gelu else AF.Square
                        nc.scalar.activation(hs[:, t, :], hps, gf,
                                             bias=b1[:, t:t + 1])
                    # W2 accumulate over j
                    for t, mps in ((0, mlp_ps0), (1, mlp_ps1)):
                        for s in range(2):
                            nc.tensor.matmul(
                                mps, W2b[:, s, t * 128:(t + 1) * 128], hs[:, s, :],
                                start=(j == 0 and s == 0), stop=(j == 11 and s == 1),
                                skip_group_check=True)

                if stage < 50:
                    continue
                # ---- reduce over kk; + 48*b2 ----
                pw = blk.tile([128, 2, 128], bf16, tag="pw")
                with nc.allow_low_precision(reason="k-sum of 4 into bf16 for Wl rhs"):
                    for t, mps in ((0, mlp_ps0), (1, mlp_ps1)):
                        nc.vector.tensor_reduce(
                            pw[:, t, :], mps[:, :].rearrange("p (k i) -> p i k", k=4),
                            mybir.AxisListType.X, OP.add)
                        nc.vector.tensor_scalar(pw[:, t, :], pw[:, t, :], b2x[:, t:t + 1], None, OP.add)

                # ---- Wl matmuls + LN2 ----
                rsl = slice(b * BLK, (b + 1) * BLK)
                y2 = psS.tile([128, 128], fp32, tag="small")
                nc.tensor.matmul(y2, Wlb[:, 0, :], pw[:, 0, :], start=True, stop=False)
                nc.tensor.matmul(y2, Wlb[:, 1, :], pw[:, 1, :], start=False, stop=False)
                nc.tensor.matmul(y2, Wlb[0:32, 2, :], ex_allb[:, rsl], start=False, stop=True)
                sq2 = blk.tile([128, 128], bf16, tag="sq2")
                nc.scalar.activation(sq2, y2, AF.Square)
                srep2 = psS.tile([128, 512], fp32, tag="small")
                nc.tensor.matmul(srep2[:, 0:128], ones_kb, sq2, start=True, stop=True)
                rs2 = blk.tile([128, 128], bf16, tag="rs2")
                nc.scalar.activation(rs2, srep2[:, 0:128], AF.Abs_reciprocal_sqrt,
                                     scale=1.0 / PAIR, bias=epsT)
                loc = blk.tile([128, 128], fp32, tag="loc")
                nc.vector.scalar_tensor_tensor(loc, y2, ln2g, rs2, OP.mult, OP.mult)
                nc.vector.tensor_scalar(loc, loc, ln2b, None, OP.add)
                nc.sync.dma_start(out=local_out[:, rsl], in_=loc)

    nc.finalize()
    RPC, NBLK = RPC_SAVE, NBLK_SAVE
    return nc


def _host_prep(inputs):
    pos = np.asarray(inputs["positions"])[:, 1].astype(np.float32)      # [N,3]
    is_aa = np.asarray(inputs["is_aa_int"]).astype(bool)
    chain = np.asarray(inputs["chain_index"]).astype(np.int64)

    aa_idx = np.where(is_aa)[0]
    sm_idx = np.where(~is_aa)[0]
    n_aa, n_sm = len(aa_idx), len(sm_idx)
    NAAP = _ceil_to(n_aa, 512)
    NSMP = _ceil_to(n_sm, 512)
    NT = NAAP + NSMP

    perm = np.concatenate([aa_idx, sm_idx])            # row r -> orig atom
    col2orig = np.full(NT, 0, np.int64)
    col2orig[:n_aa] = aa_idx
    col2orig[NAAP:NAAP + n_sm] = sm_idx

    table = np.full((NT, 3), 1.0e4, np.float32)
    table[:n_aa] = pos[aa_idx]
    table[NAAP:NAAP + n_sm] = pos[sm_idx]
    npad_a = NAAP - n_aa
    table[n_aa:NAAP, 0] += 64.0 * np.arange(1, npad_a + 1, dtype=np.float32)
    npad_s = NSMP - n_sm
    table[NAAP + n_sm:, 1] += 64.0 * np.arange(1, npad_s + 1, dtype=np.float32)

    row_col = np.empty(N, np.float32)                  # row -> its table col
    row_col[:n_aa] = np.arange(n_aa)
    row_col[n_aa:] = NAAP + np.arange(n_sm)

    ch_of_row = chain[perm]
    ch_aa = chain[aa_idx]; ch_sm = chain[sm_idx]
    nch = int(chain.max()) + 1
    aa_st = np.searchsorted(ch_aa, np.arange(nch + 1))
    sm_st = np.searchsorted(ch_sm, np.arange(nch + 1))
    c = ch_of_row
    lo_aa = aa_st[c].astype(np.float32); hi_aa = aa_st[c + 1].astype(np.float32)
    lo_sm = (NAAP + sm_st[c]).astype(np.float32); hi_sm = (NAAP + sm_st[c + 1]).astype(np.float32)

    return dict(NT=NT, NAAP=NAAP, perm=perm, col2orig=col2orig, table=table,
                row_col=row_col, lo_aa=lo_aa, hi_aa=hi_aa, lo_sm=lo_sm, hi_sm=hi_sm)


def kernel(_trace=False, _trace_dir=None, **inputs):
    import concourse.bass_utils as bass_utils

    hp = _host_prep(inputs)
    NT, NAAP = hp["NT"], hp["NAAP"]
    perm = hp["perm"]

    key = (NT, NAAP)
    if key not in _CACHE:
        _CACHE[key] = _build_graph(NT, NAAP)
    nc = _CACHE[key]

    NTv = hp["NT"]
    posT = np.concatenate([
        np.ascontiguousarray(hp["table"].T),
        np.ones((1, NTv), np.float32),
        np.zeros((1, NTv), np.float32)], axis=0)               # [5, NT]
    pos = np.asarray(inputs["positions"])[:, 1].astype(np.float32)
    aa = np.asarray(inputs["aa"]).astype(np.float32)
    isaa_f = np.asarray(inputs["is_aa_int"]).astype(np.float32)

    Wp_p = np.zeros((32, PAIR), np.float32)
    Wp_p[:19] = np.asarray(inputs["Wp"], np.float32)
    Wl_full = np.asarray(inputs["Wl"], np.float32)
    Wl_p = np.zeros((288, PAIR), np.float32)
    Wl_p[:256] = Wl_full[:256]
    Wl_p[256:277] = Wl_full[257:278]      # one_hot rows first
    Wl_p[277] = Wl_full[256]              # then is_aa row

    in_maps = []
    for c in range(NCORES):
        rs = slice(c * RPC, (c + 1) * RPC)
        rows = perm[rs]
        in_maps.append({
            "pos_rows": np.concatenate([
                np.ascontiguousarray(pos[rows].T),
                np.zeros((1, RPC), np.float32),
                np.full((1, RPC), -1.0, np.float32)], axis=0),
            "posT": posT,
            "own_col": hp["row_col"][rs],
            "ch_lo_aa": hp["lo_aa"][rs], "ch_hi_aa": hp["hi_aa"][rs],
            "ch_lo_sm": hp["lo_sm"][rs], "ch_hi_sm": hp["hi_sm"][rs],
            "aa_row": aa[rows], "isaa_row": isaa_f[rows],
            "Wp": Wp_p, "W1": np.asarray(inputs["W1"], np.float32),
            "W2": np.asarray(inputs["W2"], np.float32), "Wl": Wl_p,
            "ln1_g": np.asarray(inputs["ln1_g"], np.float32),
            "ln1_b": np.asarray(inputs["ln1_b"], np.float32),
            "ln2_g": np.asarray(inputs["ln2_g"], np.float32),
            "ln2_b": np.asarray(inputs["ln2_b"], np.float32),
            "b1": np.asarray(inputs["b1"], np.float32),
            "b2": np.asarray(inputs["b2"], np.float32),
            "iota128": np.arange(128, dtype=np.float32),
            "iota21v": np.concatenate([np.arange(21), np.zeros(11)]).astype(np.float32),
        })

    kw = {}
    if _trace:
        kw = dict(trace=True, tmpdir=_trace_dir)
    res = bass_utils.run_bass_kernel_spmd(nc, in_maps, core_ids=list(range(NCORES)), **kw)
    if _trace:
        print("HW exec time:", res.exec_time_ns, "ns")
    results = res.results

    # ---- host unshard / unpermute ----
    local = np.empty((N, PAIR), np.float32)
    pair = np.empty((N, KTOT, PAIR), np.float32)
    nbrs = np.empty((N, KTOT), np.int32)

    for c in range(NCORES):
        r = results[c]
        rows = perm[c * RPC:(c + 1) * RPC]
        local[rows] = np.asarray(r["local_out"]).T
        po = np.asarray(r["pair_out"]).reshape(PAIR, NBLK, 12, 4, 128)
        po = po.transpose(1, 4, 2, 3, 0).reshape(RPC, KTOT, PAIR)
        pair[rows] = po
        nb = np.asarray(r["nbr_out"]).astype(np.int64)
        nb[:, K_AA:] += NAAP
        nbrs[rows] = hp["col2orig"][nb]

    mask = np.asarray(inputs["all_atom_mask"])[:, 1] > 0
    return (local, pair, nbrs.astype(np.int32), np.asarray(mask))


# revision 32
# speedup vs baseline: 1.9363x; 1.1575x over previous
"""AtomPairEmbedding Trainium2 kernel — 8-core SPMD, row-parallel.

Strategy (hardcoded for N=8192, K=32+16, PAIR=128, LOCAL=256):
 - Host glue: stable-permute atoms so aa atoms occupy table cols [0, n_aa_pad)
   and small molecules [n_aa_pad, NT); pad each category with far-away
   sentinels to a multiple of 512. Shard rows (permuted order) 1024/core.
 - Device per 128-row block:
     * PE computes  -d2 = 2*dot - sq_i - sq_j  via a K=5 matmul over the
       full table (NT/512 tiles) into PSUM; ScalarE evacuates to SBUF (V).
     * VectorE: per-128-col-segment max8 (top-8 per segment), then a
       max8/match_replace ladder on the segment candidates per category,
       with max_index against V to recover global column ids.
     * Edge features (16 RBF + type + same_residue + other_chain) built with
       tiny [128,48] ops; no gathers: type/chain/residue features derive
       from column ranges (category-contiguous layout + sorted chains).
     * Feature tile transposed via PE, then feature-on-partition MLP:
       pair = LN(edge @ Wp); h = gelu(pair@W1+b1); mlp=h@W2 accumulated in
       PSUM over all 48 neighbours; local = LN(local_in @ Wl).
 - Host glue out: un-permute rows, map table cols back to original atom ids.
"""

import numpy as np

N = 8192
K_AA, K_SMOL = 32, 16
KTOT = K_AA + K_SMOL
PAIR = 128
LOCAL = 256
RBF_BINS = 16
EPS = 1e-5
NCORES = 8
RPC = N // NCORES          # rows per core
BLK = 128                  # rows per block
NBLK = RPC // BLK
NEG_BIG = -3.0e38

_CACHE = {}


def _ceil_to(x, m):
    return (x + m - 1) // m * m


def _build_graph(NT, NAAP, nblk=NBLK, rpc=RPC, use_gelu=True, stage=50, d2_dtype=None):
    """Build the single-core Bass graph (SPMD across 8 cores)."""
    import concourse.bacc as bacc
    import concourse.mybir as mybir
    from concourse.tile import TileContext

    fp32 = mybir.dt.float32
    bf16 = mybir.dt.bfloat16
    u32 = mybir.dt.uint32
    i32 = mybir.dt.int32
    AF = mybir.ActivationFunctionType
    OP = mybir.AluOpType

    global RPC, NBLK
    RPC_SAVE, NBLK_SAVE = RPC, NBLK
    RPC, NBLK = rpc, nblk
    NSEG = NT // 128               # segments of 128
    SEG_AA = NAAP // 128
    NT_TILES = NT // 512
    R_AA = K_AA // 8               # merge rounds aa
    R_SM = K_SMOL // 8

    d2dt = {None: mybir.dt.float32, "f32r": mybir.dt.float32r}[d2_dtype]
    nc = bacc.Bacc()

    # ---- DRAM parameters (per-core values via in_maps) ----
    pos_rows = nc.declare_dram_parameter("pos_rows", [5, RPC], fp32, isOutput=False)
    posT = nc.declare_dram_parameter("posT", [5, NT], fp32, isOutput=False)
    own_col = nc.declare_dram_parameter("own_col", [RPC], fp32, isOutput=False)
    ch_lo_aa = nc.declare_dram_parameter("ch_lo_aa", [RPC], fp32, isOutput=False)
    ch_hi_aa = nc.declare_dram_parameter("ch_hi_aa", [RPC], fp32, isOutput=False)
    ch_lo_sm = nc.declare_dram_parameter("ch_lo_sm", [RPC], fp32, isOutput=False)
    ch_hi_sm = nc.declare_dram_parameter("ch_hi_sm", [RPC], fp32, isOutput=False)
    aa_row = nc.declare_dram_parameter("aa_row", [RPC], fp32, isOutput=False)
    isaa_row = nc.declare_dram_parameter("isaa_row", [RPC], fp32, isOutput=False)
    Wp_d = nc.declare_dram_parameter("Wp", [32, PAIR], fp32, isOutput=False)
    W1_d = nc.declare_dram_parameter("W1", [PAIR, 2 * PAIR], fp32, isOutput=False)
    W2_d = nc.declare_dram_parameter("W2", [2 * PAIR, LOCAL], fp32, isOutput=False)
    Wl_d = nc.declare_dram_parameter("Wl", [288, PAIR], fp32, isOutput=False)
    ln1g_d = nc.declare_dram_parameter("ln1_g", [PAIR], fp32, isOutput=False)
    ln1b_d = nc.declare_dram_parameter("ln1_b", [PAIR], fp32, isOutput=False)
    ln2g_d = nc.declare_dram_parameter("ln2_g", [PAIR], fp32, isOutput=False)
    ln2b_d = nc.declare_dram_parameter("ln2_b", [PAIR], fp32, isOutput=False)
    b1_d = nc.declare_dram_parameter("b1", [2 * PAIR], fp32, isOutput=False)
    b2_d = nc.declare_dram_parameter("b2", [LOCAL], fp32, isOutput=False)
    io128_d = nc.declare_dram_parameter("iota128", [128], fp32, isOutput=False)
    io21_d = nc.declare_dram_parameter("iota21v", [32], fp32, isOutput=False)

    pair_out = nc.declare_dram_parameter(
        "pair_out", [PAIR, NBLK * 12 * 512], fp32, isOutput=True)
    local_out = nc.declare_dram_parameter("local_out", [PAIR, RPC], fp32, isOutput=True)
    nbr_out = nc.declare_dram_parameter("nbr_out", [RPC, KTOT], u32, isOutput=True)

    with TileContext(nc) as tc:
        with tc.tile_pool(name="setup", bufs=1) as setup, \
             tc.tile_pool(name="blk", bufs=2) as blk, \
             tc.tile_pool(name="vpool", bufs=2) as vpool, \
             tc.tile_pool(name="mlpbuf", bufs=1) as mlpbuf, \
             tc.tile_pool(name="psA", bufs=3, space="PSUM") as psA, \
             tc.tile_pool(name="psS", bufs=2, space="PSUM") as psS, \
             tc.tile_pool(name="accps", bufs=1, space="PSUM") as accps:

            # ---------- constants ----------
            ones_r = setup.tile([1, 128], fp32, tag="ones_r")
            nc.vector.memset(ones_r, 1.0)
            ident = setup.tile([128, 128], fp32, tag="ident")
            iota_col_f = setup.tile([128, 1], fp32, tag="iocf")
            iota_row_f = setup.tile([128, 128], fp32, tag="iorf")
            nc.sync.dma_start(out=iota_col_f, in_=io128_d[:].rearrange("(p o) -> p o", o=1))
            nc.sync.dma_start(out=iota_row_f[0:1, :], in_=io128_d[:].rearrange("(o n) -> o n", o=1))
            iops = psA.tile([128, 512], fp32, tag="ps")
            nc.tensor.matmul(iops[:, 0:128], ones_r, iota_row_f[0:1, :], start=True, stop=True)
            nc.scalar.copy(iota_row_f, iops[:, 0:128])
            nc.vector.tensor_scalar(ident, iota_row_f, iota_col_f, None, OP.is_equal)


            ones_k = setup.tile([128, 1], fp32, tag="ones_k")       # reduce over 128 parts
            nc.vector.memset(ones_k, 1.0)
            ones_kb = setup.tile([128, 128], bf16, tag="ones_kb")   # replicate-sum lhsT
            nc.vector.memset(ones_kb, 1.0)
            epsT = setup.tile([128, 1], fp32, tag="epsT")
            nc.vector.memset(epsT, EPS)


            # ---------- weights / vectors ----------
            Wp4 = setup.tile([128, 4, PAIR], fp32, tag="Wp4")
            nc.vector.memset(Wp4, 0.0)
            nc.sync.dma_start(out=Wp4[0:32, 0, :], in_=Wp_d[:, :])
            W1 = setup.tile([128, 2 * PAIR], fp32, tag="W1")
            nc.sync.dma_start(out=W1, in_=W1_d[:, :])
            W2 = setup.tile([128, 2, LOCAL], fp32, tag="W2")
            nc.sync.dma_start(out=W2, in_=W2_d[:, :].rearrange("(t p) n -> p t n", p=128))
            Wl = setup.tile([128, 3, PAIR], fp32, tag="Wl")
            nc.sync.dma_start(out=Wl[:, 0, :], in_=Wl_d[0:128, :])
            nc.sync.dma_start(out=Wl[:, 1, :], in_=Wl_d[128:256, :])
            nc.sync.dma_start(out=Wl[0:32, 2, :], in_=Wl_d[256:288, :])
            Wp4b = setup.tile([128, 4, PAIR], bf16, tag="Wp4b")
            W1b = setup.tile([128, 2 * PAIR], bf16, tag="W1b")
            W2b = setup.tile([128, 2, LOCAL], bf16, tag="W2b")
            Wlb = setup.tile([128, 3, PAIR], bf16, tag="Wlb")
            nc.vector.tensor_copy(W1b, W1)
            nc.vector.tensor_copy(W2b, W2)

            ln1g = setup.tile([128, 1], fp32, tag="ln1g")
            ln1b = setup.tile([128, 1], fp32, tag="ln1b")
            ln2g = setup.tile([128, 1], fp32, tag="ln2g")
            ln2b = setup.tile([128, 1], fp32, tag="ln2b")
            nc.sync.dma_start(out=ln1g, in_=ln1g_d[:].rearrange("(p o) -> p o", o=1))
            nc.sync.dma_start(out=ln1b, in_=ln1b_d[:].rearrange("(p o) -> p o", o=1))
            nc.sync.dma_start(out=ln2g, in_=ln2g_d[:].rearrange("(p o) -> p o", o=1))
            nc.sync.dma_start(out=ln2b, in_=ln2b_d[:].rearrange("(p o) -> p o", o=1))
            b1 = setup.tile([128, 2], fp32, tag="b1")
            nc.sync.dma_start(out=b1, in_=b1_d[:].rearrange("(t p) -> p t", p=128))
            b2x = setup.tile([128, 2], fp32, tag="b2x")
            nc.sync.dma_start(out=b2x, in_=b2_d[:].rearrange("(t p) -> p t", p=128))
            nc.vector.tensor_scalar(b2x, b2x, float(KTOT), None, OP.mult)

            # center Wp / Wl columns (free axis) so LN mean-subtract is free
            wpm = setup.tile([32, 1], fp32, tag="wpm")
            nc.vector.tensor_reduce(wpm, Wp4[0:32, 0, :], mybir.AxisListType.X, OP.add)
            nc.vector.tensor_scalar(wpm, wpm, 1.0 / PAIR, None, OP.mult)
            nc.vector.tensor_scalar(Wp4[0:32, 0, :], Wp4[0:32, 0, :], wpm, None, OP.subtract)
            for q in range(1, 4):
                nc.sync.dma_start(out=Wp4[q * 32:(q + 1) * 32, q, :], in_=Wp4[0:32, 0, :])
            nc.vector.tensor_copy(Wp4b, Wp4)
            for t in range(3):
                pr = 128 if t < 2 else 32
                wlm = setup.tile([128, 1], fp32, tag="wlm")
                nc.vector.tensor_reduce(wlm[:pr], Wl[:pr, t, :], mybir.AxisListType.X, OP.add)
                nc.vector.tensor_scalar(wlm[:pr], wlm[:pr], 1.0 / PAIR, None, OP.mult)
                nc.vector.tensor_scalar(Wl[:pr, t, :], Wl[:pr, t, :], wlm[:pr], None, OP.subtract)
            nc.vector.tensor_copy(Wlb, Wl)

            # extras table [32, RPC]: row0 = is_aa, rows 1..21 = one_hot(aa)
            ex_all = setup.tile([32, RPC], fp32, tag="ex_all")
            nc.vector.memset(ex_all, 0.0)
            aarow_sb = setup.tile([1, RPC], fp32, tag="aarow_sb")
            nc.sync.dma_start(out=aarow_sb, in_=aa_row[:].rearrange("(o n) -> o n", o=1))
            for t0 in range(0, RPC, 512):
                w = min(512, RPC - t0)
                sl = slice(t0, t0 + w)
                bps = psA.tile([128, 512], fp32, tag="ps")
                nc.tensor.matmul(bps[0:21, 0:w], ones_r[:, 0:21], aarow_sb[:, sl],
                                 start=True, stop=True)
                nc.scalar.copy(ex_all[0:21, sl], bps[0:21, 0:w])
            nc.sync.dma_start(out=ex_all[21:22, :], in_=isaa_row[:].rearrange("(o n) -> o n", o=1))
            iota21 = setup.tile([32, 1], fp32, tag="io21")
            nc.sync.dma_start(out=iota21, in_=io21_d[:].rearrange("(p o) -> p o", o=1))
            nc.vector.tensor_scalar(ex_all[0:21, :], ex_all[0:21, :], iota21[0:21], None, OP.is_equal)
            ex_allb = setup.tile([32, RPC], bf16, tag="ex_allb")
            nc.vector.tensor_copy(ex_allb, ex_all)

            # per-row scalars, laid out [128, NBLK]
            rowsc = {}
            for nm, dv in (("own", own_col), ("laa", ch_lo_aa), ("haa", ch_hi_aa),
                           ("lsm", ch_lo_sm), ("hsm", ch_hi_sm)):
                t_ = setup.tile([128, NBLK], fp32, tag="rs_" + nm)
                nc.sync.dma_start(out=t_, in_=dv[:].rearrange("(b p) -> p b", p=128))
                rowsc[nm] = t_

            # ---------- rhs table [5, NT]: 2-> posT, ones, sq ----------
            rhs = setup.tile([5, NT], d2dt, tag="rhs")
            nc.gpsimd.dma_start(out=rhs, in_=posT[:, :])
            possq = vpool.tile([128, NT], fp32, tag="V")   # reuse V storage
            nc.vector.tensor_mul(possq[0:3, :], rhs[0:3, :], rhs[0:3, :])
            ones3 = setup.tile([3, 1], fp32, tag="ones3")
            nc.vector.memset(ones3, 1.0)
            sqtab_t = vpool.tile([128, NT], fp32, tag="V")
            sqtab = sqtab_t[0:1, :]
            for t in range(NT_TILES):
                sl = slice(t * 512, (t + 1) * 512)
                sq_ps = psA.tile([128, 512], fp32, tag="ps")
                nc.tensor.matmul(sq_ps[0:1, :], ones3, possq[0:3, sl], start=True, stop=True)
                nc.scalar.copy(sqtab[:, sl], sq_ps[0:1, :])
            nc.gpsimd.dma_start(out=rhs[4:5, :], in_=sqtab)

            # ---------- lhsT [5, RPC]: 2*pos, -sq_i, -1 ----------
            lhsT = setup.tile([5, RPC], d2dt, tag="lhsT")
            nc.gpsimd.dma_start(out=lhsT, in_=pos_rows[:, :])
            nc.vector.tensor_scalar(lhsT[0:3, :], lhsT[0:3, :], 2.0, None, OP.mult)
            rsq = setup.tile([3, RPC], fp32, tag="rsq")
            nc.sync.dma_start(out=rsq, in_=pos_rows[0:3, :])
            nc.vector.tensor_mul(rsq, rsq, rsq)
            negones3 = setup.tile([3, 1], fp32, tag="negones3")
            nc.vector.memset(negones3, -1.0)
            sqi_tmp = setup.tile([1, RPC], fp32, tag="sqi_tmp")
            for t0 in range(0, RPC, 512):
                w = min(512, RPC - t0)
                sl = slice(t0, t0 + w)
                sq_ps2 = psA.tile([128, 512], fp32, tag="ps")
                nc.tensor.matmul(sq_ps2[0:1, 0:w], negones3, rsq[:, sl], start=True, stop=True)
                nc.scalar.copy(sqi_tmp[:, sl], sq_ps2[0:1, 0:w])
            # move row (partition 0) to partition 3 of lhsT via sbuf-sbuf DMA
            nc.gpsimd.dma_start(out=lhsT[3:4, :], in_=sqi_tmp)

            # =====================================================
            # per-block processing
            # =====================================================
            prev_ln2 = None
            for b in range(NBLK):
                lb = lhsT[:, b * BLK:(b + 1) * BLK]

                # ---- -d2 + evacuation ----
                V = vpool.tile([128, NT], fp32, tag="V")
                for t in range(NT_TILES):
                    sl = slice(t * 512, (t + 1) * 512)
                    ps = psA.tile([128, 512], fp32, tag="ps")
                    nc.tensor.matmul(ps, lb, rhs[:, sl], start=True, stop=True)
                    nc.scalar.copy(V[:, sl], ps)

                if stage < 2:
                    continue
                # ---- seg top8 ----
                S = blk.tile([128, NSEG * 8], fp32, tag="S")
                for s in range(NSEG):
                    nc.vector.max(S[:, s * 8:(s + 1) * 8], V[:, s * 128:(s + 1) * 128])

                # ---- merge ladders ----
                Wv = blk.tile([128, KTOT], fp32, tag="Wv")
                Gu = blk.tile([128, KTOT], u32, tag="Gu")
                Saa = S[:, :SEG_AA * 8]
                Ssm = S[:, SEG_AA * 8:NSEG * 8]
                Vaa = V[:, :NAAP]
                Vsm = V[:, NAAP:NT]
                for r in range(R_AA):
                    w8 = Wv[:, r * 8:(r + 1) * 8]
                    nc.vector.max(w8, Saa)
                    nc.vector.max_index(Gu[:, r * 8:(r + 1) * 8], w8, Vaa)
                    if r + 1 < R_AA:
                        nc.vector.match_replace(Saa, w8, Saa, NEG_BIG)
                for r in range(R_SM):
                    w8 = Wv[:, K_AA + r * 8:K_AA + (r + 1) * 8]
                    nc.vector.max(w8, Ssm)
                    nc.vector.max_index(Gu[:, K_AA + r * 8:K_AA + (r + 1) * 8], w8, Vsm)
                    if r + 1 < R_SM:
                        nc.vector.match_replace(Ssm, w8, Ssm, NEG_BIG)

                nc.sync.dma_start(out=nbr_out[b * BLK:(b + 1) * BLK, :], in_=Gu)

                if stage < 3:
                    continue
                # ---- features [128, 48] ----
                Gf = blk.tile([128, KTOT], fp32, tag="Gf")
                nc.vector.tensor_copy(Gf, Gu)
                nc.vector.tensor_scalar(Gf[:, K_AA:], Gf[:, K_AA:], float(NAAP), None, OP.add)

                d2p = blk.tile([128, KTOT], fp32, tag="d2p")
                nc.vector.tensor_scalar(d2p, Wv, -1.0, 1e-12, OP.mult, OP.max)
                dd = blk.tile([128, KTOT], fp32, tag="dd")
                nc.scalar.activation(dd, d2p, AF.Ln)
                nc.scalar.activation(dd, dd, AF.Exp, scale=0.5)

                sc = blk.tile([128, KTOT], fp32, tag="sc")
                m2 = blk.tile([128, KTOT], fp32, tag="m2")
                bsl = slice(b, b + 1)
                nc.vector.tensor_scalar(sc[:, :K_AA], Gf[:, :K_AA], rowsc["laa"][:, bsl], None, OP.is_ge)
                nc.vector.tensor_scalar(m2[:, :K_AA], Gf[:, :K_AA], rowsc["haa"][:, bsl], None, OP.is_lt)
                nc.vector.tensor_scalar(sc[:, K_AA:], Gf[:, K_AA:], rowsc["lsm"][:, bsl], None, OP.is_ge)
                nc.vector.tensor_scalar(m2[:, K_AA:], Gf[:, K_AA:], rowsc["hsm"][:, bsl], None, OP.is_lt)
                nc.vector.tensor_mul(sc, sc, m2)
                oc = blk.tile([128, KTOT], fp32, tag="oc")
                nc.vector.tensor_scalar(oc, sc, -1.0, 1.0, OP.mult, OP.add)
                sr = blk.tile([128, KTOT], fp32, tag="sr")
                nc.vector.tensor_scalar(sr, Gf, rowsc["own"][:, bsl], None, OP.is_equal)

                # rbf: scratch r-major [16 x 48], squared, exp into F strided
                F = blk.tile([128, KTOT * 32], fp32, tag="F")
                nc.vector.memset(F, 0.0)
                Rt = blk.tile([128, RBF_BINS * KTOT], fp32, tag="Rt")
                sig = (22.0 - 2.0) / RBF_BINS
                for r_ in range(RBF_BINS):
                    c_r = 2.0 + r_ * (20.0 / (RBF_BINS - 1))
                    nc.vector.tensor_scalar(
                        Rt[:, r_ * KTOT:(r_ + 1) * KTOT], dd, -c_r, None, OP.add)
                nc.vector.tensor_mul(Rt, Rt, Rt)
                Fv = F[:, :].rearrange("p (k f) -> p k f", f=32)
                Rv = Rt[:, :].rearrange("p (f k) -> p k f", k=KTOT)
                nc.scalar.activation(Fv[:, :, 0:RBF_BINS], Rv, AF.Exp,
                                     scale=-1.0 / (sig * sig))
                nc.vector.memset(Fv[:, 0:K_AA, 16:17], 1.0)
                nc.vector.tensor_copy(Fv[:, :, 17:18], sr[:, :].rearrange("p (k o) -> p k o", o=1))
                nc.vector.tensor_copy(Fv[:, :, 18:19], oc[:, :].rearrange("p (k o) -> p k o", o=1))

                if stage < 35:
                    continue
                # ---- transpose F -> FT [128, 12, 128] ----
                FT = blk.tile([128, 12, 128], bf16, tag="FT")
                for j in range(12):
                    tps = psA.tile([128, 512], fp32, tag="ps")
                    nc.tensor.transpose(tps[:, 0:128], F[:, j * 128:(j + 1) * 128], ident)
                    nc.scalar.copy(FT[:, j, :], tps[:, 0:128])

                if stage < 40:
                    continue
                # ---- MLP, phase-split to batch ACT table sets ----
                mlp_ps0 = accps.tile([128, 512], fp32, tag="mlpa")
                mlp_ps1 = accps.tile([128, 512], fp32, tag="mlpb")
                praw = mlpbuf.tile([128, 12, 512], bf16, tag="praw")
                sigb = mlpbuf.tile([128, 12, 512], bf16, tag="sigb")

                # phase 1: Wp matmuls + squares + per-token ssq (filler-func ACT only)
                for j in range(12):
                    pairps = psA.tile([128, 512], fp32, tag="ps")
                    for kk in range(4):
                        nc.tensor.matmul(
                            pairps[:, kk * 128:(kk + 1) * 128],
                            Wp4b[:, kk, :],
                            FT[:, j, :], start=True, stop=True)
                    sqt = blk.tile([128, 512], bf16, tag="sqt")
                    nc.scalar.activation(sqt, pairps, AF.Square)
                    srep = psS.tile([128, 512], fp32, tag="small")
                    nc.tensor.matmul(srep, ones_kb, sqt, start=True, stop=True)
                    nc.scalar.copy(sigb[:, j, :], srep)
                    nc.scalar.copy(praw[:, j, :], pairps)
                # phase 2: one abs_rsqrt over the whole block (+ deferred LN2 of prev)
                rsgb = sigb
                nc.scalar.activation(rsgb, sigb, AF.Abs_reciprocal_sqrt,
                                     scale=1.0 / PAIR, bias=epsT)
                if prev_ln2 is not None:
                    py2, ps2, prsl = prev_ln2
                    prs2 = blk.tile([128, 128], bf16, tag="rs2")
                    nc.scalar.activation(prs2, ps2, AF.Abs_reciprocal_sqrt,
                                         scale=1.0 / PAIR, bias=epsT)
                    loc = blk.tile([128, 128], fp32, tag="loc")
                    nc.vector.scalar_tensor_tensor(loc, py2, ln2g, prs2, OP.mult, OP.mult)
                    nc.vector.tensor_scalar(loc, loc, ln2b, None, OP.add)
                    nc.sync.dma_start(out=local_out[:, prsl], in_=loc)
                    prev_ln2 = None
                # phase 3: normalize + W1/gelu/W2 (gelu set only)
                for j in range(12):
                    pair_b = blk.tile([128, 512], bf16, tag="pair_sb")
                    nc.vector.scalar_tensor_tensor(
                        pair_b, praw[:, j, :], ln1g, rsgb[:, j, :], OP.mult, OP.mult)
                    nc.vector.tensor_scalar(pair_b, pair_b, ln1b, None, OP.add)
                    pair_f = blk.tile([128, 512], fp32, tag="pair_f")
                    nc.vector.tensor_copy(pair_f, pair_b)
                    nc.sync.dma_start(
                        out=pair_out[:, (b * 12 + j) * 512:(b * 12 + j + 1) * 512],
                        in_=pair_f)
                    hs = blk.tile([128, 2, 512], bf16, tag="hs")
                    for t in range(2):
                        hps = psA.tile([128, 512], fp32, tag="ps")
                        nc.tensor.matmul(hps, W1b[:, t * 128:(t + 1) * 128], pair_b,
                                         start=True, stop=True)
                        gf = AF.Gelu_apprx_tanh if use_gelu else AF.Square
                        nc.scalar.activation(hs[:, t, :], hps, gf,
                                             bias=b1[:, t:t + 1])
                    for t, mps in ((0, mlp_ps0), (1, mlp_ps1)):
                        for s in range(2):
                            nc.tensor.matmul(
                                mps, W2b[:, s, t * 128:(t + 1) * 128], hs[:, s, :],
                                start=(j == 0 and s == 0), stop=(j == 11 and s == 1),
                                skip_group_check=True)

                if stage < 50:
                    continue
                # ---- reduce over kk; + 48*b2 ----
                pw = blk.tile([128, 2, 128], bf16, tag="pw")
                with nc.allow_low_precision(reason="k-sum of 4 into bf16 for Wl rhs"):
                    for t, mps in ((0, mlp_ps0), (1, mlp_ps1)):
                        nc.vector.tensor_reduce(
                            pw[:, t, :], mps[:, :].rearrange("p (k i) -> p i k", k=4),
                            mybir.AxisListType.X, OP.add)
                        nc.vector.tensor_scalar(pw[:, t, :], pw[:, t, :], b2x[:, t:t + 1], None, OP.add)

                # ---- Wl matmuls + LN2 ----
                rsl = slice(b * BLK, (b + 1) * BLK)
                y2 = psS.tile([128, 128], fp32, tag="small")
                nc.tensor.matmul(y2, Wlb[:, 0, :], pw[:, 0, :], start=True, stop=False)
                nc.tensor.matmul(y2, Wlb[:, 1, :], pw[:, 1, :], start=False, stop=False)
                nc.tensor.matmul(y2, Wlb[0:32, 2, :], ex_allb[:, rsl], start=False, stop=True)
                sq2 = blk.tile([128, 128], bf16, tag="sq2")
                nc.scalar.activation(sq2, y2, AF.Square)
                srep2 = psS.tile([128, 512], fp32, tag="small")
                nc.tensor.matmul(srep2[:, 0:128], ones_kb, sq2, start=True, stop=True)
                y2sb = blk.tile([128, 128], bf16, tag="y2sb")
                nc.scalar.copy(y2sb, y2)
                s2b = blk.tile([128, 128], bf16, tag="s2b")
                nc.scalar.copy(s2b, srep2[:, 0:128])
                prev_ln2 = (y2sb, s2b, rsl)

            if prev_ln2 is not None:
                py2, ps2, prsl = prev_ln2
                prs2 = blk.tile([128, 128], bf16, tag="rs2")
                nc.scalar.activation(prs2, ps2, AF.Abs_reciprocal_sqrt,
                                     scale=1.0 / PAIR, bias=epsT)
                loc = blk.tile([128, 128], fp32, tag="loc")
                nc.vector.scalar_tensor_tensor(loc, py2, ln2g, prs2, OP.mult, OP.mult)
                nc.vector.tensor_scalar(loc, loc, ln2b, None, OP.add)
                nc.sync.dma_start(out=local_out[:, prsl], in_=loc)

    nc.finalize()
    RPC, NBLK = RPC_SAVE, NBLK_SAVE
    return nc


def _host_prep(inputs):
    pos = np.asarray(inputs["positions"])[:, 1].astype(np.float32)      # [N,3]
    is_aa = np.asarray(inputs["is_aa_int"]).astype(bool)
    chain = np.asarray(inputs["chain_index"]).astype(np.int64)

    aa_idx = np.where(is_aa)[0]
    sm_idx = np.where(~is_aa)[0]
    n_aa, n_sm = len(aa_idx), len(sm_idx)
    NAAP = _ceil_to(n_aa, 512)
    NSMP = _ceil_to(n_sm, 512)
    NT = NAAP + NSMP

    perm = np.concatenate([aa_idx, sm_idx])            # row r -> orig atom
    col2orig = np.full(NT, 0, np.int64)
    col2orig[:n_aa] = aa_idx
    col2orig[NAAP:NAAP + n_sm] = sm_idx

    table = np.full((NT, 3), 1.0e4, np.float32)
    table[:n_aa] = pos[aa_idx]
    table[NAAP:NAAP + n_sm] = pos[sm_idx]
    npad_a = NAAP - n_aa
    table[n_aa:NAAP, 0] += 64.0 * np.arange(1, npad_a + 1, dtype=np.float32)
    npad_s = NSMP - n_sm
    table[NAAP + n_sm:, 1] += 64.0 * np.arange(1, npad_s + 1, dtype=np.float32)

    row_col = np.empty(N, np.float32)                  # row -> its table col
    row_col[:n_aa] = np.arange(n_aa)
    row_col[n_aa:] = NAAP + np.arange(n_sm)

    ch_of_row = chain[perm]
    ch_aa = chain[aa_idx]; ch_sm = chain[sm_idx]
    nch = int(chain.max()) + 1
    aa_st = np.searchsorted(ch_aa, np.arange(nch + 1))
    sm_st = np.searchsorted(ch_sm, np.arange(nch + 1))
    c = ch_of_row
    lo_aa = aa_st[c].astype(np.float32); hi_aa = aa_st[c + 1].astype(np.float32)
    lo_sm = (NAAP + sm_st[c]).astype(np.float32); hi_sm = (NAAP + sm_st[c + 1]).astype(np.float32)

    return dict(NT=NT, NAAP=NAAP, perm=perm, col2orig=col2orig, table=table,
                row_col=row_col, lo_aa=lo_aa, hi_aa=hi_aa, lo_sm=lo_sm, hi_sm=hi_sm)


def kernel(_trace=False, _trace_dir=None, **inputs):
    import concourse.bass_utils as bass_utils

    hp = _host_prep(inputs)
    NT, NAAP = hp["NT"], hp["NAAP"]
    perm = hp["perm"]

    key = (NT, NAAP)
    if key not in _CACHE:
        _CACHE[key] = _build_graph(NT, NAAP)
    nc = _CACHE[key]

    NTv = hp["NT"]
    posT = np.concatenate([
        np.ascontiguousarray(hp["table"].T),
        np.ones((1, NTv), np.float32),
        np.zeros((1, NTv), np.float32)], axis=0)               # [5, NT]
    pos = np.asarray(inputs["positions"])[:, 1].astype(np.float32)
    aa = np.asarray(inputs["aa"]).astype(np.float32)
    isaa_f = np.asarray(inputs["is_aa_int"]).astype(np.float32)

    Wp_p = np.zeros((32, PAIR), np.float32)
    Wp_p[:19] = np.asarray(inputs["Wp"], np.float32)
    Wl_full = np.asarray(inputs["Wl"], np.float32)
    Wl_p = np.zeros((288, PAIR), np.float32)
    Wl_p[:256] = Wl_full[:256]
    Wl_p[256:277] = Wl_full[257:278]      # one_hot rows first
    Wl_p[277] = Wl_full[256]              # then is_aa row

    in_maps = []
    for c in range(NCORES):
        rs = slice(c * RPC, (c + 1) * RPC)
        rows = perm[rs]
        in_maps.append({
            "pos_rows": np.concatenate([
                np.ascontiguousarray(pos[rows].T),
                np.zeros((1, RPC), np.float32),
                np.full((1, RPC), -1.0, np.float32)], axis=0),
            "posT": posT,
            "own_col": hp["row_col"][rs],
            "ch_lo_aa": hp["lo_aa"][rs], "ch_hi_aa": hp["hi_aa"][rs],
            "ch_lo_sm": hp["lo_sm"][rs], "ch_hi_sm": hp["hi_sm"][rs],
            "aa_row": aa[rows], "isaa_row": isaa_f[rows],
            "Wp": Wp_p, "W1": np.asarray(inputs["W1"], np.float32),
            "W2": np.asarray(inputs["W2"], np.float32), "Wl": Wl_p,
            "ln1_g": np.asarray(inputs["ln1_g"], np.float32),
            "ln1_b": np.asarray(inputs["ln1_b"], np.float32),
            "ln2_g": np.asarray(inputs["ln2_g"], np.float32),
            "ln2_b": np.asarray(inputs["ln2_b"], np.float32),
            "b1": np.asarray(inputs["b1"], np.float32),
            "b2": np.asarray(inputs["b2"], np.float32),
            "iota128": np.arange(128, dtype=np.float32),
            "iota21v": np.concatenate([np.arange(21), np.zeros(11)]).astype(np.float32),
        })

    kw = {}
    if _trace:
        kw = dict(trace=True, tmpdir=_trace_dir)
    res = bass_utils.run_bass_kernel_spmd(nc, in_maps, core_ids=list(range(NCORES)), **kw)
    if _trace:
        print("HW exec time:", res.exec_time_ns, "ns")
    results = res.results

    # ---- host unshard / unpermute ----
    local = np.empty((N, PAIR), np.float32)
    pair = np.empty((N, KTOT, PAIR), np.float32)
    nbrs = np.empty((N, KTOT), np.int32)

    for c in range(NCORES):
        r = results[c]
        rows = perm[c * RPC:(c + 1) * RPC]
        local[rows] = np.asarray(r["local_out"]).T
        po = np.asarray(r["pair_out"]).reshape(PAIR, NBLK, 12, 4, 128)
        po = po.transpose(1, 4, 2, 3, 0).reshape(RPC, KTOT, PAIR)
        pair[rows] = po
        nb = np.asarray(r["nbr_out"]).astype(np.int64)
        nb[:, K_AA:] += NAAP
        nbrs[rows] = hp["col2orig"][nb]

    mask = np.asarray(inputs["all_atom_mask"])[:, 1] > 0
    return (local, pair, nbrs.astype(np.int32), np.asarray(mask))


# revision 33
# speedup vs baseline: 2.7596x; 1.4252x over previous
"""AtomPairEmbedding Trainium2 kernel — 8-core SPMD, row-parallel.

Strategy (hardcoded for N=8192, K=32+16, PAIR=128, LOCAL=256):
 - Host glue: stable-permute atoms so aa atoms occupy table cols [0, n_aa_pad)
   and small molecules [n_aa_pad, NT); pad each category with far-away
   sentinels to a multiple of 512. Shard rows (permuted order) 1024/core.
 - Device per 128-row block:
     * PE computes  -d2 = 2*dot - sq_i - sq_j  via a K=5 matmul over the
       full table (NT/512 tiles) into PSUM; ScalarE evacuates to SBUF (V).
     * VectorE: per-128-col-segment max8 (top-8 per segment), then a
       max8/match_replace ladder on the segment candidates per category,
       with max_index against V to recover global column ids.
     * Edge features (16 RBF + type + same_residue + other_chain) built with
       tiny [128,48] ops; no gathers: type/chain/residue features derive
       from column ranges (category-contiguous layout + sorted chains).
     * Feature tile transposed via PE, then feature-on-partition MLP:
       pair = LN(edge @ Wp); h = gelu(pair@W1+b1); mlp=h@W2 accumulated in
       PSUM over all 48 neighbours; local = LN(local_in @ Wl).
 - Host glue out: un-permute rows, map table cols back to original atom ids.
"""

import numpy as np

N = 8192
K_AA, K_SMOL = 32, 16
KTOT = K_AA + K_SMOL
PAIR = 128
LOCAL = 256
RBF_BINS = 16
EPS = 1e-5
NCORES = 8
RPC = N // NCORES          # rows per core
BLK = 128                  # rows per block
NBLK = RPC // BLK
NEG_BIG = -3.0e38

_CACHE = {}


def _ceil_to(x, m):
    return (x + m - 1) // m * m


def _build_graph(NT, NAAP, nblk=NBLK, rpc=RPC, use_gelu=True, stage=50, d2_dtype=None):
    """Build the single-core Bass graph (SPMD across 8 cores)."""
    import concourse.bacc as bacc
    import concourse.mybir as mybir
    from concourse.tile import TileContext

    fp32 = mybir.dt.float32
    bf16 = mybir.dt.bfloat16
    u32 = mybir.dt.uint32
    i32 = mybir.dt.int32
    AF = mybir.ActivationFunctionType
    OP = mybir.AluOpType

    global RPC, NBLK
    RPC_SAVE, NBLK_SAVE = RPC, NBLK
    RPC, NBLK = rpc, nblk
    NSEG = NT // 128               # segments of 128
    SEG_AA = NAAP // 128
    NT_TILES = NT // 512
    R_AA = K_AA // 8               # merge rounds aa
    R_SM = K_SMOL // 8

    d2dt = {None: mybir.dt.float32, "f32r": mybir.dt.float32r}[d2_dtype]
    nc = bacc.Bacc()

    # ---- DRAM parameters (per-core values via in_maps) ----
    pos_rows = nc.declare_dram_parameter("pos_rows", [5, RPC], fp32, isOutput=False)
    posT = nc.declare_dram_parameter("posT", [5, NT], fp32, isOutput=False)
    own_col = nc.declare_dram_parameter("own_col", [RPC], fp32, isOutput=False)
    ch_lo_aa = nc.declare_dram_parameter("ch_lo_aa", [RPC], fp32, isOutput=False)
    ch_hi_aa = nc.declare_dram_parameter("ch_hi_aa", [RPC], fp32, isOutput=False)
    ch_lo_sm = nc.declare_dram_parameter("ch_lo_sm", [RPC], fp32, isOutput=False)
    ch_hi_sm = nc.declare_dram_parameter("ch_hi_sm", [RPC], fp32, isOutput=False)
    aa_row = nc.declare_dram_parameter("aa_row", [RPC], fp32, isOutput=False)
    isaa_row = nc.declare_dram_parameter("isaa_row", [RPC], fp32, isOutput=False)
    Wp_d = nc.declare_dram_parameter("Wp", [32, PAIR], fp32, isOutput=False)
    W1_d = nc.declare_dram_parameter("W1", [PAIR, 2 * PAIR], fp32, isOutput=False)
    W2_d = nc.declare_dram_parameter("W2", [2 * PAIR, LOCAL], fp32, isOutput=False)
    Wl_d = nc.declare_dram_parameter("Wl", [288, PAIR], fp32, isOutput=False)
    ln1g_d = nc.declare_dram_parameter("ln1_g", [PAIR], fp32, isOutput=False)
    ln1b_d = nc.declare_dram_parameter("ln1_b", [PAIR], fp32, isOutput=False)
    ln2g_d = nc.declare_dram_parameter("ln2_g", [PAIR], fp32, isOutput=False)
    ln2b_d = nc.declare_dram_parameter("ln2_b", [PAIR], fp32, isOutput=False)
    b1_d = nc.declare_dram_parameter("b1", [2 * PAIR], fp32, isOutput=False)
    b2_d = nc.declare_dram_parameter("b2", [LOCAL], fp32, isOutput=False)
    io128_d = nc.declare_dram_parameter("iota128", [128], fp32, isOutput=False)
    io21_d = nc.declare_dram_parameter("iota21v", [32], fp32, isOutput=False)

    pair_out = nc.declare_dram_parameter(
        "pair_out", [PAIR, NBLK * 12 * 512], fp32, isOutput=True)
    local_out = nc.declare_dram_parameter("local_out", [PAIR, RPC], fp32, isOutput=True)
    nbr_out = nc.declare_dram_parameter("nbr_out", [RPC, KTOT], u32, isOutput=True)

    with TileContext(nc) as tc:
        with tc.tile_pool(name="setup", bufs=1) as setup, \
             tc.tile_pool(name="blk", bufs=2) as blk, \
             tc.tile_pool(name="vpool", bufs=2) as vpool, \
             tc.tile_pool(name="mlpbuf", bufs=1) as mlpbuf, \
             tc.tile_pool(name="psA", bufs=3, space="PSUM") as psA, \
             tc.tile_pool(name="d2ps", bufs=2, space="PSUM") as d2ps, \
             tc.tile_pool(name="psS", bufs=1, space="PSUM") as psS, \
             tc.tile_pool(name="accps", bufs=1, space="PSUM") as accps:

            # ---------- constants ----------
            ones_r = setup.tile([1, 128], fp32, tag="ones_r")
            nc.vector.memset(ones_r, 1.0)
            ident = setup.tile([128, 128], fp32, tag="ident")
            iota_col_f = setup.tile([128, 1], fp32, tag="iocf")
            iota_row_f = setup.tile([128, 128], fp32, tag="iorf")
            nc.sync.dma_start(out=iota_col_f, in_=io128_d[:].rearrange("(p o) -> p o", o=1))
            nc.sync.dma_start(out=iota_row_f[0:1, :], in_=io128_d[:].rearrange("(o n) -> o n", o=1))
            iops = psA.tile([128, 512], fp32, tag="ps")
            nc.tensor.matmul(iops[:, 0:128], ones_r, iota_row_f[0:1, :], start=True, stop=True)
            nc.scalar.copy(iota_row_f, iops[:, 0:128])
            nc.vector.tensor_scalar(ident, iota_row_f, iota_col_f, None, OP.is_equal)


            ones_k = setup.tile([128, 1], fp32, tag="ones_k")       # reduce over 128 parts
            nc.vector.memset(ones_k, 1.0)
            ones_kb = setup.tile([128, 128], bf16, tag="ones_kb")   # replicate-sum lhsT
            nc.vector.memset(ones_kb, 1.0)
            epsT = setup.tile([128, 1], fp32, tag="epsT")
            nc.vector.memset(epsT, EPS)


            # ---------- weights / vectors ----------
            Wp4 = setup.tile([128, 4, PAIR], fp32, tag="Wp4")
            nc.vector.memset(Wp4, 0.0)
            nc.sync.dma_start(out=Wp4[0:32, 0, :], in_=Wp_d[:, :])
            W1 = setup.tile([128, 2 * PAIR], fp32, tag="W1")
            nc.sync.dma_start(out=W1, in_=W1_d[:, :])
            W2 = setup.tile([128, 2, LOCAL], fp32, tag="W2")
            nc.sync.dma_start(out=W2, in_=W2_d[:, :].rearrange("(t p) n -> p t n", p=128))
            Wl = setup.tile([128, 3, PAIR], fp32, tag="Wl")
            nc.sync.dma_start(out=Wl[:, 0, :], in_=Wl_d[0:128, :])
            nc.sync.dma_start(out=Wl[:, 1, :], in_=Wl_d[128:256, :])
            nc.sync.dma_start(out=Wl[0:32, 2, :], in_=Wl_d[256:288, :])
            Wp4b = setup.tile([128, 4, PAIR], bf16, tag="Wp4b")
            W1b = setup.tile([128, 2 * PAIR], bf16, tag="W1b")
            W2b = setup.tile([128, 2, LOCAL], bf16, tag="W2b")
            Wlb = setup.tile([128, 3, PAIR], bf16, tag="Wlb")
            nc.vector.tensor_copy(W1b, W1)
            nc.vector.tensor_copy(W2b, W2)

            ln1g = setup.tile([128, 1], fp32, tag="ln1g")
            ln1b = setup.tile([128, 1], fp32, tag="ln1b")
            ln2g = setup.tile([128, 1], fp32, tag="ln2g")
            ln2b = setup.tile([128, 1], fp32, tag="ln2b")
            nc.sync.dma_start(out=ln1g, in_=ln1g_d[:].rearrange("(p o) -> p o", o=1))
            nc.sync.dma_start(out=ln1b, in_=ln1b_d[:].rearrange("(p o) -> p o", o=1))
            nc.sync.dma_start(out=ln2g, in_=ln2g_d[:].rearrange("(p o) -> p o", o=1))
            nc.sync.dma_start(out=ln2b, in_=ln2b_d[:].rearrange("(p o) -> p o", o=1))
            b1 = setup.tile([128, 2], fp32, tag="b1")
            nc.sync.dma_start(out=b1, in_=b1_d[:].rearrange("(t p) -> p t", p=128))
            b2x = setup.tile([128, 2], fp32, tag="b2x")
            nc.sync.dma_start(out=b2x, in_=b2_d[:].rearrange("(t p) -> p t", p=128))
            nc.vector.tensor_scalar(b2x, b2x, float(KTOT), None, OP.mult)

            # center Wp / Wl columns (free axis) so LN mean-subtract is free
            wpm = setup.tile([32, 1], fp32, tag="wpm")
            nc.vector.tensor_reduce(wpm, Wp4[0:32, 0, :], mybir.AxisListType.X, OP.add)
            nc.vector.tensor_scalar(wpm, wpm, 1.0 / PAIR, None, OP.mult)
            nc.vector.tensor_scalar(Wp4[0:32, 0, :], Wp4[0:32, 0, :], wpm, None, OP.subtract)
            for q in range(1, 4):
                nc.sync.dma_start(out=Wp4[q * 32:(q + 1) * 32, q, :], in_=Wp4[0:32, 0, :])
            nc.vector.tensor_copy(Wp4b, Wp4)
            for t in range(3):
                pr = 128 if t < 2 else 32
                wlm = setup.tile([128, 1], fp32, tag="wlm")
                nc.vector.tensor_reduce(wlm[:pr], Wl[:pr, t, :], mybir.AxisListType.X, OP.add)
                nc.vector.tensor_scalar(wlm[:pr], wlm[:pr], 1.0 / PAIR, None, OP.mult)
                nc.vector.tensor_scalar(Wl[:pr, t, :], Wl[:pr, t, :], wlm[:pr], None, OP.subtract)
            nc.vector.tensor_copy(Wlb, Wl)

            # extras table [32, RPC]: row0 = is_aa, rows 1..21 = one_hot(aa)
            ex_all = setup.tile([32, RPC], fp32, tag="ex_all")
            nc.vector.memset(ex_all, 0.0)
            aarow_sb = setup.tile([1, RPC], fp32, tag="aarow_sb")
            nc.sync.dma_start(out=aarow_sb, in_=aa_row[:].rearrange("(o n) -> o n", o=1))
            for t0 in range(0, RPC, 512):
                w = min(512, RPC - t0)
                sl = slice(t0, t0 + w)
                bps = psA.tile([128, 512], fp32, tag="ps")
                nc.tensor.matmul(bps[0:21, 0:w], ones_r[:, 0:21], aarow_sb[:, sl],
                                 start=True, stop=True)
                nc.scalar.copy(ex_all[0:21, sl], bps[0:21, 0:w])
            nc.sync.dma_start(out=ex_all[21:22, :], in_=isaa_row[:].rearrange("(o n) -> o n", o=1))
            iota21 = setup.tile([32, 1], fp32, tag="io21")
            nc.sync.dma_start(out=iota21, in_=io21_d[:].rearrange("(p o) -> p o", o=1))
            nc.vector.tensor_scalar(ex_all[0:21, :], ex_all[0:21, :], iota21[0:21], None, OP.is_equal)
            ex_allb = setup.tile([32, RPC], bf16, tag="ex_allb")
            nc.vector.tensor_copy(ex_allb, ex_all)

            # per-row scalars, laid out [128, NBLK]
            rowsc = {}
            for nm, dv in (("own", own_col), ("laa", ch_lo_aa), ("haa", ch_hi_aa),
                           ("lsm", ch_lo_sm), ("hsm", ch_hi_sm)):
                t_ = setup.tile([128, NBLK], fp32, tag="rs_" + nm)
                nc.sync.dma_start(out=t_, in_=dv[:].rearrange("(b p) -> p b", p=128))
                rowsc[nm] = t_

            # ---------- rhs table [5, NT]: 2-> posT, ones, sq ----------
            rhs = setup.tile([5, NT], d2dt, tag="rhs")
            nc.gpsimd.dma_start(out=rhs, in_=posT[:, :])
            possq = vpool.tile([128, NT], fp32, tag="V")   # reuse V storage
            nc.vector.tensor_mul(possq[0:3, :], rhs[0:3, :], rhs[0:3, :])
            ones3 = setup.tile([3, 1], fp32, tag="ones3")
            nc.vector.memset(ones3, 1.0)
            sqtab_t = vpool.tile([128, NT], fp32, tag="V")
            sqtab = sqtab_t[0:1, :]
            for t in range(NT_TILES):
                sl = slice(t * 512, (t + 1) * 512)
                sq_ps = psA.tile([128, 512], fp32, tag="ps")
                nc.tensor.matmul(sq_ps[0:1, :], ones3, possq[0:3, sl], start=True, stop=True)
                nc.scalar.copy(sqtab[:, sl], sq_ps[0:1, :])
            nc.gpsimd.dma_start(out=rhs[4:5, :], in_=sqtab)

            # ---------- lhsT [5, RPC]: 2*pos, -sq_i, -1 ----------
            lhsT = setup.tile([5, RPC], d2dt, tag="lhsT")
            nc.gpsimd.dma_start(out=lhsT, in_=pos_rows[:, :])
            nc.vector.tensor_scalar(lhsT[0:3, :], lhsT[0:3, :], 2.0, None, OP.mult)
            rsq = setup.tile([3, RPC], fp32, tag="rsq")
            nc.sync.dma_start(out=rsq, in_=pos_rows[0:3, :])
            nc.vector.tensor_mul(rsq, rsq, rsq)
            negones3 = setup.tile([3, 1], fp32, tag="negones3")
            nc.vector.memset(negones3, -1.0)
            sqi_tmp = setup.tile([1, RPC], fp32, tag="sqi_tmp")
            for t0 in range(0, RPC, 512):
                w = min(512, RPC - t0)
                sl = slice(t0, t0 + w)
                sq_ps2 = psA.tile([128, 512], fp32, tag="ps")
                nc.tensor.matmul(sq_ps2[0:1, 0:w], negones3, rsq[:, sl], start=True, stop=True)
                nc.scalar.copy(sqi_tmp[:, sl], sq_ps2[0:1, 0:w])
            # move row (partition 0) to partition 3 of lhsT via sbuf-sbuf DMA
            nc.gpsimd.dma_start(out=lhsT[3:4, :], in_=sqi_tmp)

            # =====================================================
            # per-block processing
            # =====================================================
            prev_ln2 = None
            for b in range(NBLK):
                lb = lhsT[:, b * BLK:(b + 1) * BLK]

                # ---- -d2 + evacuation ----
                V = vpool.tile([128, NT], fp32, tag="V")
                for t in range(NT_TILES):
                    sl = slice(t * 512, (t + 1) * 512)
                    ps = d2ps.tile([128, 512], fp32, tag="d2")
                    nc.tensor.matmul(ps, lb, rhs[:, sl], start=True, stop=True)
                    if t % 2 == 0:
                        nc.scalar.copy(V[:, sl], ps)
                    else:
                        nc.vector.tensor_copy(V[:, sl], ps)

                if stage < 2:
                    continue
                # ---- seg top8 ----
                S = blk.tile([128, NSEG * 8], fp32, tag="S")
                for s in range(NSEG):
                    nc.vector.max(S[:, s * 8:(s + 1) * 8], V[:, s * 128:(s + 1) * 128])

                # ---- merge ladders ----
                Wv = blk.tile([128, KTOT], fp32, tag="Wv")
                Gu = blk.tile([128, KTOT], u32, tag="Gu")
                Saa = S[:, :SEG_AA * 8]
                Ssm = S[:, SEG_AA * 8:NSEG * 8]
                Vaa = V[:, :NAAP]
                Vsm = V[:, NAAP:NT]
                for r in range(R_AA):
                    w8 = Wv[:, r * 8:(r + 1) * 8]
                    nc.vector.max(w8, Saa)
                    nc.vector.max_index(Gu[:, r * 8:(r + 1) * 8], w8, Vaa)
                    if r + 1 < R_AA:
                        nc.vector.match_replace(Saa, w8, Saa, NEG_BIG)
                for r in range(R_SM):
                    w8 = Wv[:, K_AA + r * 8:K_AA + (r + 1) * 8]
                    nc.vector.max(w8, Ssm)
                    nc.vector.max_index(Gu[:, K_AA + r * 8:K_AA + (r + 1) * 8], w8, Vsm)
                    if r + 1 < R_SM:
                        nc.vector.match_replace(Ssm, w8, Ssm, NEG_BIG)

                nc.sync.dma_start(out=nbr_out[b * BLK:(b + 1) * BLK, :], in_=Gu)

                if stage < 3:
                    continue
                # ---- features [128, 48] ----
                Gf = blk.tile([128, KTOT], fp32, tag="Gf")
                nc.vector.tensor_copy(Gf, Gu)
                nc.vector.tensor_scalar(Gf[:, K_AA:], Gf[:, K_AA:], float(NAAP), None, OP.add)

                d2p = blk.tile([128, KTOT], fp32, tag="d2p")
                nc.vector.tensor_scalar(d2p, Wv, -1.0, 1e-12, OP.mult, OP.max)
                dd = blk.tile([128, KTOT], fp32, tag="dd")
                nc.scalar.activation(dd, d2p, AF.Ln)
                nc.scalar.activation(dd, dd, AF.Exp, scale=0.5)

                sc = blk.tile([128, KTOT], fp32, tag="sc")
                m2 = blk.tile([128, KTOT], fp32, tag="m2")
                bsl = slice(b, b + 1)
                nc.vector.tensor_scalar(sc[:, :K_AA], Gf[:, :K_AA], rowsc["laa"][:, bsl], None, OP.is_ge)
                nc.vector.tensor_scalar(m2[:, :K_AA], Gf[:, :K_AA], rowsc["haa"][:, bsl], None, OP.is_lt)
                nc.vector.tensor_scalar(sc[:, K_AA:], Gf[:, K_AA:], rowsc["lsm"][:, bsl], None, OP.is_ge)
                nc.vector.tensor_scalar(m2[:, K_AA:], Gf[:, K_AA:], rowsc["hsm"][:, bsl], None, OP.is_lt)
                nc.vector.tensor_mul(sc, sc, m2)
                oc = blk.tile([128, KTOT], fp32, tag="oc")
                nc.vector.tensor_scalar(oc, sc, -1.0, 1.0, OP.mult, OP.add)
                sr = blk.tile([128, KTOT], fp32, tag="sr")
                nc.vector.tensor_scalar(sr, Gf, rowsc["own"][:, bsl], None, OP.is_equal)

                # rbf: scratch r-major [16 x 48], squared, exp into F strided
                F = blk.tile([128, KTOT * 32], fp32, tag="F")
                nc.vector.memset(F, 0.0)
                Rt = blk.tile([128, RBF_BINS * KTOT], fp32, tag="Rt")
                sig = (22.0 - 2.0) / RBF_BINS
                for r_ in range(RBF_BINS):
                    c_r = 2.0 + r_ * (20.0 / (RBF_BINS - 1))
                    nc.vector.tensor_scalar(
                        Rt[:, r_ * KTOT:(r_ + 1) * KTOT], dd, -c_r, None, OP.add)
                nc.vector.tensor_mul(Rt, Rt, Rt)
                Fv = F[:, :].rearrange("p (k f) -> p k f", f=32)
                Rv = Rt[:, :].rearrange("p (f k) -> p k f", k=KTOT)
                nc.scalar.activation(Fv[:, :, 0:RBF_BINS], Rv, AF.Exp,
                                     scale=-1.0 / (sig * sig))
                nc.vector.memset(Fv[:, 0:K_AA, 16:17], 1.0)
                nc.vector.tensor_copy(Fv[:, :, 17:18], sr[:, :].rearrange("p (k o) -> p k o", o=1))
                nc.vector.tensor_copy(Fv[:, :, 18:19], oc[:, :].rearrange("p (k o) -> p k o", o=1))

                if stage < 35:
                    continue
                # ---- transpose F -> FT [128, 12, 128] ----
                FT = blk.tile([128, 12, 128], bf16, tag="FT")
                for j in range(12):
                    tps = psA.tile([128, 512], fp32, tag="ps")
                    nc.tensor.transpose(tps[:, 0:128], F[:, j * 128:(j + 1) * 128], ident)
                    if j % 2 == 0:
                        nc.scalar.copy(FT[:, j, :], tps[:, 0:128])
                    else:
                        nc.vector.tensor_copy(FT[:, j, :], tps[:, 0:128])

                if stage < 40:
                    continue
                # ---- MLP, phase-split to batch ACT table sets ----
                mlp_ps0 = accps.tile([128, 512], fp32, tag="mlpa")
                mlp_ps1 = accps.tile([128, 512], fp32, tag="mlpb")
                praw = mlpbuf.tile([128, 12, 512], bf16, tag="praw")
                sigb = mlpbuf.tile([128, 12, 512], bf16, tag="sigb")

                # phase 1: Wp matmuls + squares + per-token ssq (filler-func ACT only)
                for j in range(12):
                    pairps = psA.tile([128, 512], fp32, tag="ps")
                    for kk in range(4):
                        nc.tensor.matmul(
                            pairps[:, kk * 128:(kk + 1) * 128],
                            Wp4b[:, kk, :],
                            FT[:, j, :], start=True, stop=True)
                    sqt = blk.tile([128, 512], bf16, tag="sqt")
                    nc.scalar.activation(sqt, pairps, AF.Square)
                    srep = psS.tile([128, 512], fp32, tag="small")
                    nc.tensor.matmul(srep, ones_kb, sqt, start=True, stop=True)
                    nc.scalar.copy(sigb[:, j, :], srep)
                    nc.vector.tensor_copy(praw[:, j, :], pairps)
                # phase 2: one abs_rsqrt over the whole block (+ deferred LN2 of prev)
                rsgb = sigb
                nc.scalar.activation(rsgb, sigb, AF.Abs_reciprocal_sqrt,
                                     scale=1.0 / PAIR, bias=epsT)
                if prev_ln2 is not None:
                    py2, ps2, prsl = prev_ln2
                    prs2 = blk.tile([128, 128], bf16, tag="rs2")
                    nc.scalar.activation(prs2, ps2, AF.Abs_reciprocal_sqrt,
                                         scale=1.0 / PAIR, bias=epsT)
                    loc = blk.tile([128, 128], fp32, tag="loc")
                    nc.vector.scalar_tensor_tensor(loc, py2, ln2g, prs2, OP.mult, OP.mult)
                    nc.vector.tensor_scalar(loc, loc, ln2b, None, OP.add)
                    nc.sync.dma_start(out=local_out[:, prsl], in_=loc)
                    prev_ln2 = None
                # phase 3: normalize + W1/gelu/W2 (gelu set only)
                for j in range(12):
                    pair_b = blk.tile([128, 512], bf16, tag="pair_sb")
                    nc.vector.scalar_tensor_tensor(
                        pair_b, praw[:, j, :], ln1g, rsgb[:, j, :], OP.mult, OP.mult)
                    nc.vector.tensor_scalar(pair_b, pair_b, ln1b, None, OP.add)
                    pair_f = blk.tile([128, 512], fp32, tag="pair_f")
                    nc.scalar.copy(pair_f, pair_b)
                    nc.sync.dma_start(
                        out=pair_out[:, (b * 12 + j) * 512:(b * 12 + j + 1) * 512],
                        in_=pair_f)
                    hs = blk.tile([128, 2, 512], bf16, tag="hs")
                    for t in range(2):
                        hps = psA.tile([128, 512], fp32, tag="ps")
                        nc.tensor.matmul(hps, W1b[:, t * 128:(t + 1) * 128], pair_b,
                                         start=True, stop=True)
                        gf = AF.Gelu_apprx_tanh if use_gelu else AF.Square
                        nc.scalar.activation(hs[:, t, :], hps, gf,
                                             bias=b1[:, t:t + 1])
                    for t, mps in ((0, mlp_ps0), (1, mlp_ps1)):
                        for s in range(2):
                            nc.tensor.matmul(
                                mps, W2b[:, s, t * 128:(t + 1) * 128], hs[:, s, :],
                                start=(j == 0 and s == 0), stop=(j == 11 and s == 1),
                                skip_group_check=True)

                if stage < 50:
                    continue
                # ---- reduce over kk; + 48*b2 ----
                pw = blk.tile([128, 2, 128], bf16, tag="pw")
                with nc.allow_low_precision(reason="k-sum of 4 into bf16 for Wl rhs"):
                    for t, mps in ((0, mlp_ps0), (1, mlp_ps1)):
                        nc.vector.tensor_reduce(
                            pw[:, t, :], mps[:, :].rearrange("p (k i) -> p i k", k=4),
                            mybir.AxisListType.X, OP.add)
                        nc.vector.tensor_scalar(pw[:, t, :], pw[:, t, :], b2x[:, t:t + 1], None, OP.add)

                # ---- Wl matmuls + LN2 ----
                rsl = slice(b * BLK, (b + 1) * BLK)
                y2 = psS.tile([128, 128], fp32, tag="small")
                nc.tensor.matmul(y2, Wlb[:, 0, :], pw[:, 0, :], start=True, stop=False)
                nc.tensor.matmul(y2, Wlb[:, 1, :], pw[:, 1, :], start=False, stop=False)
                nc.tensor.matmul(y2, Wlb[0:32, 2, :], ex_allb[:, rsl], start=False, stop=True)
                sq2 = blk.tile([128, 128], bf16, tag="sq2")
                nc.scalar.activation(sq2, y2, AF.Square)
                srep2 = psS.tile([128, 512], fp32, tag="small")
                nc.tensor.matmul(srep2[:, 0:128], ones_kb, sq2, start=True, stop=True)
                y2sb = blk.tile([128, 128], bf16, tag="y2sb")
                nc.scalar.copy(y2sb, y2)
                s2b = blk.tile([128, 128], bf16, tag="s2b")
                nc.scalar.copy(s2b, srep2[:, 0:128])
                prev_ln2 = (y2sb, s2b, rsl)

            if prev_ln2 is not None:
                py2, ps2, prsl = prev_ln2
                prs2 = blk.tile([128, 128], bf16, tag="rs2")
                nc.scalar.activation(prs2, ps2, AF.Abs_reciprocal_sqrt,
                                     scale=1.0 / PAIR, bias=epsT)
                loc = blk.tile([128, 128], fp32, tag="loc")
                nc.vector.scalar_tensor_tensor(loc, py2, ln2g, prs2, OP.mult, OP.mult)
                nc.vector.tensor_scalar(loc, loc, ln2b, None, OP.add)
                nc.sync.dma_start(out=local_out[:, prsl], in_=loc)

    nc.finalize()
    RPC, NBLK = RPC_SAVE, NBLK_SAVE
    return nc


def _host_prep(inputs):
    pos = np.asarray(inputs["positions"])[:, 1].astype(np.float32)      # [N,3]
    is_aa = np.asarray(inputs["is_aa_int"]).astype(bool)
    chain = np.asarray(inputs["chain_index"]).astype(np.int64)

    aa_idx = np.where(is_aa)[0]
    sm_idx = np.where(~is_aa)[0]
    n_aa, n_sm = len(aa_idx), len(sm_idx)
    NAAP = _ceil_to(n_aa, 512)
    NSMP = _ceil_to(n_sm, 512)
    NT = NAAP + NSMP

    perm = np.concatenate([aa_idx, sm_idx])            # row r -> orig atom
    col2orig = np.full(NT, 0, np.int64)
    col2orig[:n_aa] = aa_idx
    col2orig[NAAP:NAAP + n_sm] = sm_idx

    table = np.full((NT, 3), 1.0e4, np.float32)
    table[:n_aa] = pos[aa_idx]
    table[NAAP:NAAP + n_sm] = pos[sm_idx]
    npad_a = NAAP - n_aa
    table[n_aa:NAAP, 0] += 64.0 * np.arange(1, npad_a + 1, dtype=np.float32)
    npad_s = NSMP - n_sm
    table[NAAP + n_sm:, 1] += 64.0 * np.arange(1, npad_s + 1, dtype=np.float32)

    row_col = np.empty(N, np.float32)                  # row -> its table col
    row_col[:n_aa] = np.arange(n_aa)
    row_col[n_aa:] = NAAP + np.arange(n_sm)

    ch_of_row = chain[perm]
    ch_aa = chain[aa_idx]; ch_sm = chain[sm_idx]
    nch = int(chain.max()) + 1
    aa_st = np.searchsorted(ch_aa, np.arange(nch + 1))
    sm_st = np.searchsorted(ch_sm, np.arange(nch + 1))
    c = ch_of_row
    lo_aa = aa_st[c].astype(np.float32); hi_aa = aa_st[c + 1].astype(np.float32)
    lo_sm = (NAAP + sm_st[c]).astype(np.float32); hi_sm = (NAAP + sm_st[c + 1]).astype(np.float32)

    return dict(NT=NT, NAAP=NAAP, perm=perm, col2orig=col2orig, table=table,
                row_col=row_col, lo_aa=lo_aa, hi_aa=hi_aa, lo_sm=lo_sm, hi_sm=hi_sm)


def kernel(_trace=False, _trace_dir=None, **inputs):
    import concourse.bass_utils as bass_utils

    hp = _host_prep(inputs)
    NT, NAAP = hp["NT"], hp["NAAP"]
    perm = hp["perm"]

    key = (NT, NAAP)
    if key not in _CACHE:
        _CACHE[key] = _build_graph(NT, NAAP)
    nc = _CACHE[key]

    NTv = hp["NT"]
    posT = np.concatenate([
        np.ascontiguousarray(hp["table"].T),
        np.ones((1, NTv), np.float32),
        np.zeros((1, NTv), np.float32)], axis=0)               # [5, NT]
    pos = np.asarray(inputs["positions"])[:, 1].astype(np.float32)
    aa = np.asarray(inputs["aa"]).astype(np.float32)
    isaa_f = np.asarray(inputs["is_aa_int"]).astype(np.float32)

    Wp_p = np.zeros((32, PAIR), np.float32)
    Wp_p[:19] = np.asarray(inputs["Wp"], np.float32)
    Wl_full = np.asarray(inputs["Wl"], np.float32)
    Wl_p = np.zeros((288, PAIR), np.float32)
    Wl_p[:256] = Wl_full[:256]
    Wl_p[256:277] = Wl_full[257:278]      # one_hot rows first
    Wl_p[277] = Wl_full[256]              # then is_aa row

    in_maps = []
    for c in range(NCORES):
        rs = slice(c * RPC, (c + 1) * RPC)
        rows = perm[rs]
        in_maps.append({
            "pos_rows": np.concatenate([
                np.ascontiguousarray(pos[rows].T),
                np.zeros((1, RPC), np.float32),
                np.full((1, RPC), -1.0, np.float32)], axis=0),
            "posT": posT,
            "own_col": hp["row_col"][rs],
            "ch_lo_aa": hp["lo_aa"][rs], "ch_hi_aa": hp["hi_aa"][rs],
            "ch_lo_sm": hp["lo_sm"][rs], "ch_hi_sm": hp["hi_sm"][rs],
            "aa_row": aa[rows], "isaa_row": isaa_f[rows],
            "Wp": Wp_p, "W1": np.asarray(inputs["W1"], np.float32),
            "W2": np.asarray(inputs["W2"], np.float32), "Wl": Wl_p,
            "ln1_g": np.asarray(inputs["ln1_g"], np.float32),
            "ln1_b": np.asarray(inputs["ln1_b"], np.float32),
            "ln2_g": np.asarray(inputs["ln2_g"], np.float32),
            "ln2_b": np.asarray(inputs["ln2_b"], np.float32),
            "b1": np.asarray(inputs["b1"], np.float32),
            "b2": np.asarray(inputs["b2"], np.float32),
            "iota128": np.arange(128, dtype=np.float32),
            "iota21v": np.concatenate([np.arange(21), np.zeros(11)]).astype(np.float32),
        })

    kw = {}
    if _trace:
        kw = dict(trace=True, tmpdir=_trace_dir)
    res = bass_utils.run_bass_kernel_spmd(nc, in_maps, core_ids=list(range(NCORES)), **kw)
    if _trace:
        print("HW exec time:", res.exec_time_ns, "ns")
    results = res.results

    # ---- host unshard / unpermute ----
    local = np.empty((N, PAIR), np.float32)
    pair = np.empty((N, KTOT, PAIR), np.float32)
    nbrs = np.empty((N, KTOT), np.int32)

    for c in range(NCORES):
        r = results[c]
        rows = perm[c * RPC:(c + 1) * RPC]
        local[rows] = np.asarray(r["local_out"]).T
        po = np.asarray(r["pair_out"]).reshape(PAIR, NBLK, 12, 4, 128)
        po = po.transpose(1, 4, 2, 3, 0).reshape(RPC, KTOT, PAIR)
        pair[rows] = po
        nb = np.asarray(r["nbr_out"]).astype(np.int64)
        nb[:, K_AA:] += NAAP
        nbrs[rows] = hp["col2orig"][nb]

    mask = np.asarray(inputs["all_atom_mask"])[:, 1] > 0
    return (local, pair, nbrs.astype(np.int32), np.asarray(mask))
